# revision 30
# baseline (speedup 1.0000x reference)
"""Trainium2 Bass kernel for nn_BPRMF (segment_reduce): gather -> running-mean
-> BatchNorm(train) -> LIF spiking recurrence -> scores matmul.

Sharding over 8 NeuronCores:
  - gather/cumsum/BN/LIF: data-parallel over batch (64 rows/core); BN batch
    stats via AllReduce; LIF output exchanged via direct core-to-core
    remote-DMA broadcasts (SBUF->peer SBUF, no DRAM round trip).
  - scores matmul + output: vocab-sharded (12800 item columns/core).

Self-contained: hardcodes shapes, builds/compiles the Bass program on first
call, caches it for the process lifetime.
"""
import sys

sys.path.insert(0, "/opt/trn_rl_repo")

import numpy as np
import ml_dtypes

N_ITEMS = 100001
D = 128
T = 50
B = 512
NCORES = 8
BSH = B // NCORES          # 64 batch rows per core
VSH = 12800                # vocab shard per core (8*12800 = 102400 >= 100001)
TH = T // 2                # 25: gather packs two time-halves on 128 partitions
TAU = 2.0
V_TH = 1.0
BN_EPS = 1e-5
TSPLIT = 24                # stats A/B split (multiple of 8)
A_LAUNCH_J = 16            # gather step after which AR-A is launched on Pool
WSLOT = BSH + 1            # 65: uo payload = 64 data cols + 1 tag col

_CACHE = {}
LAST_EXEC_NS = None
LAST_RESULTS = None
DEBUG_DUMP = False
COMM_MODE = "cc"   # "cc": collective AllGather exchange; "rdma": direct peer-SBUF


class _Comm:
    """Cross-core exchange state for one build (sems + deferred waits)."""

    def __init__(self, nc):
        self.rsem = nc.alloc_semaphore(name="uo_rsem")
        self.lsem = nc.alloc_semaphore(name="uo_lsem")
        # (engine, target_inst_name, sem, value) to insert post-scheduling
        self.post_waits = []
        self.barrier_sem = nc._bir_kernel_barrier_sem
        self.barrier_val = nc.bir_kernel_barrier_sem_inc


def _attach_post_waits(nc, post_waits):
    """Insert standalone engine sem-wait instructions directly before their
    target instructions, after Tile scheduling (whose simulator cannot model
    remotely-incremented semaphores)."""
    fn = nc.m.functions[0]
    for eng, target, sem, val in post_waits:
        w = eng.wait_ge(sem, val)
        wname = w.ins.name
        wobj = None
        for blk in fn.blocks:
            insts = blk.instructions
            names = [i.name for i in insts]
            if wname in names:
                wobj = insts[names.index(wname)]
                blk.instructions = [i for i in insts if i.name != wname]
                break
        assert wobj is not None, f"wait {wname} not found"
        placed = False
        for blk in fn.blocks:
            insts = blk.instructions
            names = [i.name for i in insts]
            if target in names:
                insts.insert(names.index(target), wobj)
                blk.instructions = insts
                placed = True
                break
        assert placed, f"target {target} not found for wait insertion"


def _emit_iteration(nc, tc, aps, cs, recv, cons, it=0, comm="rdma"):
    """Emit one full pipeline iteration. Pools are scoped to the call so an
    unrolled timing build reuses the same on-chip space serially."""
    import concourse.bass as bass
    from concourse import mybir
    from contextlib import ExitStack

    f32 = mybir.dt.float32
    bf16 = mybir.dt.bfloat16
    i32 = mybir.dt.int32
    u32 = mybir.dt.uint32
    Alu = mybir.AluOpType
    Act = mybir.ActivationFunctionType

    emb, embT, offs, rdiag, pp, i128, out, perm = (
        aps["emb"], aps["embT"], aps["offs"], aps["rdiag"], aps["pp"],
        aps["i128"], aps["out"], aps["perm"])
    groups = [list(range(NCORES))]
    NA, NB_ = TSPLIT, T - TSPLIT

    # loop-invariant tiles, loaded once before iteration 0 (reloading them
    # per iteration adds WAR edges onto the previous iteration's consumers,
    # serializing the unrolled pipeline)
    offs_s, rdiag_s, pp_s, i128_s, pid_s, eps_t, embT_s = (
        cons["offs_s"], cons["rdiag_s"], cons["pp_s"], cons["i128_s"],
        cons["pid_s"], cons["eps_t"], cons["embT_s"])

    with ExitStack() as ctx:
        con = ctx.enter_context(tc.tile_pool(name=f"con{it}", bufs=1))
        work = ctx.enter_context(tc.tile_pool(name=f"work{it}", bufs=1))
        dr = ctx.enter_context(tc.tile_pool(name=f"dr{it}", bufs=1, space="DRAM"))

        # DRAM staging for the two stats AllReduces
        cc_inA = dr.tile([D, 2 * NA], f32, name=f"cc_inA{it}")
        cc_outA = dr.tile([D, 2 * NA], f32, addr_space="Shared",
                          name=f"cc_outA{it}")
        cc_inB = dr.tile([D, 2 * NB_], f32, name=f"cc_inB{it}")
        cc_outB = dr.tile([D, 2 * NB_], f32, addr_space="Shared",
                          name=f"cc_outB{it}")

        # ---- gather: offs column j holds (t=2j) on partitions p<64 and
        # (t=2j+1) on p>=64, so gather j completes ALL data for t<=2j+1. The
        # upper 64 partitions (odd t) are copied down to G2 in chunks
        # pipelined with the gather stream (tensor ops need operands at the
        # same start partition).
        G = con.tile([128, TH * D], f32, name=f"G{it}")
        G2 = con.tile([BSH, TH * D], f32, name=f"G2{it}")
        GCH = 2

        # cumsum / transpose / stats state
        pf0 = work.tile([BSH, D], f32, name=f"pf0{it}")
        pf1 = work.tile([BSH, D], f32, name=f"pf1{it}")
        pfs = [pf0, pf1]
        packed = work.tile([D, 2 * NA], f32, name=f"packed{it}")
        packedB = work.tile([D, 2 * NB_], f32, name=f"packedB{it}")
        sqbs = [work.tile([128, 8 * BSH], f32, name=f"sqb{it}_{p}")
                for p in range(2)]

        # uFT streams through 2 rotating PSUM banks into an SBUF copy: keeping
        # it fully PSUM-resident (7 banks) serializes consecutive iterations
        # against the scores matmuls' banks. Layout per iteration: uftp banks
        # 0-1, acc bank 2, psB (scores) banks 3-7 -- so iteration N+1's
        # transposes never wait on iteration N's scores.
        uftp = ctx.enter_context(tc.tile_pool(name=f"uftp{it}", bufs=2,
                                              space="PSUM"))
        accp = ctx.enter_context(tc.tile_pool(name=f"accp{it}", bufs=1,
                                              space="PSUM"))
        acc = accp.tile([128, BSH], f32, name=f"acc{it}")
        uFT_sb = con.tile([128, T * BSH], f32, name=f"uFTsb{it}")
        cur_chunk = [None]

        TPB = 8

        def stats_chunk(t0, t1):
            n = t1 - t0
            xs = cur_chunk[0][:, 0:n * BSH]
            # evict this chunk to SBUF for the later h/LIF reads
            nc.scalar.activation(uFT_sb[:, t0 * BSH:t1 * BSH], xs, Act.Copy)
            if t1 <= TSPLIT:
                dst_s = packed[:, t0:t1]
                dst_q = packed[:, NA + t0:NA + t1]
                ccdst_s = cc_inA[:, t0:t1]
                ccdst_q = cc_inA[:, NA + t0:NA + t1]
            else:
                dst_s = packedB[:, t0 - TSPLIT:t1 - TSPLIT]
                dst_q = packedB[:, NB_ + t0 - TSPLIT:NB_ + t1 - TSPLIT]
                ccdst_s = cc_inB[:, t0 - TSPLIT:t1 - TSPLIT]
                ccdst_q = cc_inB[:, NB_ + t0 - TSPLIT:NB_ + t1 - TSPLIT]
            sqb = sqbs[(t0 // TPB) % 2]
            nc.vector.tensor_reduce(
                out=dst_s, in_=xs.rearrange("p (t b) -> p t b", t=n),
                axis=mybir.AxisListType.X, op=Alu.add)
            nc.scalar.activation(sqb[:, 0:n * BSH], xs, Act.Square)
            nc.vector.tensor_reduce(
                out=dst_q,
                in_=sqb[:, 0:n * BSH].rearrange("p (t b) -> p t b", t=n),
                axis=mybir.AxisListType.X, op=Alu.add)
            # stage incrementally so the AllReduce's input is ready in DRAM
            # the moment the collective launches
            nc.sync.dma_start(ccdst_s, dst_s)
            nc.sync.dma_start(ccdst_q, dst_q)

        def emit_cumsum_steps(j):
            for t in (2 * j, 2 * j + 1):
                if t >= T:
                    continue
                if t % TPB == 0:
                    cur_chunk[0] = uftp.tile([128, TPB * BSH], f32,
                                             tag="uftc", name=f"uftc{it}_{t}")
                src = (G[0:BSH, (t // 2) * D:(t // 2 + 1) * D] if t % 2 == 0
                       else G2[0:BSH, (t // 2) * D:(t // 2 + 1) * D])
                pf = pfs[t % 2]
                if t == 0:
                    nc.vector.tensor_copy(pf[:], src)
                else:
                    nc.vector.tensor_tensor(out=pf[:], in0=pfs[(t - 1) % 2][:],
                                            in1=src, op=Alu.add)
                ti = t % TPB
                nc.tensor.matmul(cur_chunk[0][:, ti * BSH:(ti + 1) * BSH],
                                 lhsT=pf[:],
                                 rhs=rdiag_s[:, t * BSH:(t + 1) * BSH],
                                 start=True, stop=True)
                if (t + 1) % TPB == 0:
                    stats_chunk(t + 1 - TPB, t + 1)
                elif t == T - 1:
                    stats_chunk(T - T % TPB, T)

        # cumsum for a given j is only emitted once its G2 chunk's copy has
        # been emitted: Tile's dep tracking is emission-ordered, and a read
        # emitted before its writer gets no RAW edge (reads stale data).
        for j in range(TH):
            nc.gpsimd.indirect_dma_start(
                out=G[:, j * D:(j + 1) * D], out_offset=None, in_=emb,
                in_offset=bass.IndirectOffsetOnAxis(ap=offs_s[:, j:j + 1], axis=0),
            )
            if (j + 1) % GCH == 0:
                lo = (j + 1 - GCH) * D
                nc.sync.dma_start(G2[:, lo:(j + 1) * D], G[BSH:128, lo:(j + 1) * D])
                for jj in range(j + 1 - GCH, j + 1):
                    emit_cumsum_steps(jj)
            if j == A_LAUNCH_J and comm in ("rdma", "cc"):
                nc.gpsimd.collective_compute(
                    "AllReduce", Alu.add, replica_groups=groups,
                    ins=[cc_inA[:]], outs=[cc_outA[:]])
        if TH % GCH:
            lo = (TH - TH % GCH) * D
            nc.sync.dma_start(G2[:, lo:TH * D], G[BSH:128, lo:TH * D])
            for jj in range(TH - TH % GCH, TH):
                emit_cumsum_steps(jj)
        if comm in ("rdma", "cc"):
            nc.gpsimd.collective_compute(
                "AllReduce", Alu.add, replica_groups=groups,
                ins=[cc_inB[:]], outs=[cc_outB[:]])
        else:
            nc.sync.dma_start(cc_outA[:], cc_inA[:])
            nc.sync.dma_start(cc_outB[:], cc_inB[:])

        gstatsA = work.tile([D, 2 * NA], f32, name=f"gstatsA{it}")
        nc.sync.dma_start(gstatsA[:], cc_outA[:])
        gstatsB = work.tile([D, 2 * NB_], f32, name=f"gstatsB{it}")
        nc.sync.dma_start(gstatsB[:], cc_outB[:])

        # ---- BN affine params: h_t = x*s2_t + b2_t  (pre-divided by TAU)
        bh = work.tile([D, 1], f32, name=f"bh{it}")
        nc.vector.tensor_scalar(out=bh[:], in0=pp_s[:, 1:2],
                                scalar1=1.0 / TAU, scalar2=None, op0=Alu.mult)
        s2 = work.tile([D, T], f32, name=f"s2{it}")
        b2 = work.tile([D, T], f32, name=f"b2{it}")

        def emit_params(gst, n, col0, tag):
            mean = work.tile([D, n], f32, name=f"mean{tag}{it}")
            nc.vector.tensor_scalar(out=mean[:], in0=gst[:, 0:n],
                                    scalar1=1.0 / B, scalar2=None, op0=Alu.mult)
            ex2 = work.tile([D, n], f32, name=f"ex2{tag}{it}")
            nc.vector.tensor_scalar(out=ex2[:], in0=gst[:, n:2 * n],
                                    scalar1=1.0 / B, scalar2=None, op0=Alu.mult)
            var = work.tile([D, n], f32, name=f"var{tag}{it}")
            nc.vector.tensor_tensor(out=var[:], in0=mean[:], in1=mean[:],
                                    op=Alu.mult)
            nc.vector.tensor_tensor(out=var[:], in0=ex2[:], in1=var[:],
                                    op=Alu.subtract)
            std = work.tile([D, n], f32, name=f"std{tag}{it}")
            nc.scalar.activation(std[:], var[:], Act.Sqrt, bias=eps_t[:, 0:1])
            inv = work.tile([D, n], f32, name=f"inv{tag}{it}")
            nc.vector.reciprocal(inv[:], std[:])
            s2s = s2[:, col0:col0 + n]
            nc.vector.tensor_scalar(out=s2s, in0=inv[:], scalar1=pp_s[:, 0:1],
                                    scalar2=1.0 / TAU, op0=Alu.mult,
                                    op1=Alu.mult)
            ms = work.tile([D, n], f32, name=f"ms{tag}{it}")
            nc.vector.tensor_tensor(out=ms[:], in0=mean[:], in1=s2s,
                                    op=Alu.mult)
            nc.vector.scalar_tensor_tensor(
                out=b2[:, col0:col0 + n], in0=ms[:], scalar=-1.0,
                in1=bh[:, 0:1].to_broadcast((D, n)), op0=Alu.mult, op1=Alu.add)

        emit_params(gstatsA, NA, 0, "A")
        emit_params(gstatsB, NB_, TSPLIT, "B")

        # ---- LIF recurrence on the pre-reset voltage w:
        #   s_t = [w_t >= 1];  w_{t+1} = (w_t - s_t)/2 + h_{t+1}
        # h precomputed for every t into one big tile so the Act engine runs
        # ahead of the serial DVE chain; spikes accumulated on the idle PE
        # via identity-matmul PSUM accumulation (kills the tail reduce).
        h = con.tile([128, T * BSH], f32, name=f"h{it}")
        for t in range(T):
            nc.scalar.activation(h[:, t * BSH:(t + 1) * BSH],
                                 uFT_sb[:, t * BSH:(t + 1) * BSH],
                                 Act.Identity, scale=s2[:, t:t + 1],
                                 bias=b2[:, t:t + 1])

        w = work.tile([128, BSH], f32, name=f"w{it}")
        q = work.tile([128, BSH], f32, name=f"q{it}")
        spk = con.tile([128, T * BSH], f32, name=f"spk{it}")
        for t in range(T):
            ht = h[:, t * BSH:(t + 1) * BSH]
            if t == 0:
                nc.vector.tensor_copy(w[:], ht)
            else:
                nc.vector.scalar_tensor_tensor(
                    out=q[:], in0=w[:], scalar=1.0 / TAU, in1=ht,
                    op0=Alu.mult, op1=Alu.add)
                nc.vector.scalar_tensor_tensor(
                    out=w[:], in0=spk[:, (t - 1) * BSH:t * BSH],
                    scalar=-V_TH / TAU, in1=q[:], op0=Alu.mult, op1=Alu.add)
            nc.vector.tensor_scalar(out=spk[:, t * BSH:(t + 1) * BSH],
                                    in0=w[:], scalar1=V_TH, scalar2=None,
                                    op0=Alu.is_ge)
            nc.tensor.matmul(acc[:], lhsT=i128_s[:],
                             rhs=spk[:, t * BSH:(t + 1) * BSH],
                             start=(t == 0), stop=(t == T - 1))

        # uo payload [128, 65]: cols 0..63 = mean spikes (bf16), col 64 row 0
        # = this core's id tag (receivers recover the sender->slot map from it)
        uo = work.tile([128, WSLOT], bf16, name=f"uo{it}")
        uo_w = nc.scalar.activation(uo[:, 0:BSH], acc[:], Act.Identity,
                                    scale=1.0 / T)
        nc.vector.tensor_copy(uo[0:1, BSH:BSH + 1], pid_s[:])
        if comm in ("rdma", "rdma_nostats") and it > 0:
            cs.post_waits.append((nc.scalar, uo_w.ins.name, cs.lsem,
                                  16 * NCORES * it))

        # ---- uo exchange: 8 single-dest remote broadcasts (slot k -> peer
        # me^k), descriptors prepped on the idle Pool engine during the LIF,
        # fired by one trigger once uo is ready. Lands directly in peers'
        # SBUF recv tile -- no DRAM round trip, no collective.
        lhsT = work.tile([D, B], bf16, name=f"lhsT{it}")
        if comm == "cc":
            # collective AllGather fallback (v0-style): SBUF->DRAM->AG->SBUF
            ag_in = dr.tile([D, BSH], bf16, name=f"ag_in{it}")
            ag_out = dr.tile([NCORES * D, BSH], bf16, addr_space="Shared",
                             name=f"ag_out{it}")
            nc.sync.dma_start(ag_in[:], uo[:, 0:BSH])
            nc.gpsimd.collective_compute(
                "AllGather", Alu.bypass, replica_groups=groups,
                ins=[ag_in[:]], outs=[ag_out[:]])
            nc.sync.dma_start(
                lhsT[:].rearrange("p (c b) -> p c b", c=NCORES),
                ag_out[:].rearrange("(c p) b -> p c b", c=NCORES))
            for k in range(NCORES):
                nc.sync.dma_start(perm[0:1, k:k + 1], uo[0:1, BSH:BSH + 1])
        elif comm in ("rdma", "rdma_nostats"):
            for k in range(NCORES):
                rdests = [None] * NCORES
                rdests[k] = (0, k)
                nc.gpsimd.remote_dma_broadcast(
                    out_ap=recv[:, k * WSLOT:(k + 1) * WSLOT],
                    in_ap=uo[:, 0:WSLOT], remote_sem=cs.rsem,
                    local_sem=cs.lsem, rdests=rdests, queue_num=1)
            trig = nc.gpsimd.trigger_dma(count=None, queue_num=1)
            cs.post_waits.append((nc.gpsimd, trig.ins.name, cs.barrier_sem,
                                  cs.barrier_val))
            rthresh = 2 * NCORES * (it + 1)
            # assemble lhsT from the recv slots (data cols only); first copy
            # per engine carries the arrival wait
            for k in range(NCORES):
                src = recv[:, k * WSLOT:k * WSLOT + BSH]
                dst = lhsT[:, k * BSH:(k + 1) * BSH]
                if k % 2 == 0:
                    cp = nc.vector.tensor_copy(dst, src)
                    eng = nc.vector
                else:
                    cp = nc.scalar.activation(dst, src, Act.Copy)
                    eng = nc.scalar
                if k < 2:
                    cs.post_waits.append((eng, cp.ins.name, cs.rsem, rthresh))
            for k in range(NCORES):
                pdma = nc.sync.dma_start(
                    perm[0:1, k:k + 1],
                    recv[0:1, k * WSLOT + BSH:k * WSLOT + BSH + 1])
                if k == 0:
                    cs.post_waits.append((nc.sync, pdma.ins.name, cs.rsem,
                                          rthresh))
        else:
            # dev-only fallback (TimelineSim): replicate own uo into all slots
            for k in range(NCORES):
                nc.vector.tensor_copy(lhsT[:, k * BSH:(k + 1) * BSH],
                                      uo[:, 0:BSH])
                nc.sync.dma_start(perm[0:1, k:k + 1],
                                  uo[0:1, BSH:BSH + 1])

        if DEBUG_DUMP and it == 0:
            nc.sync.dma_start(aps["dbg_h"], h[:])
            nc.sync.dma_start(aps["dbg_spk"], spk[:])
            nc.sync.dma_start(aps["dbg_lhsT"], lhsT[:])
            dbg_uo_s = work.tile([128, WSLOT], mybir.dt.float32,
                                 name=f"dbg_uo_s{it}")
            nc.vector.tensor_copy(dbg_uo_s[:], uo[:])
            nc.sync.dma_start(aps["dbg_uo"], dbg_uo_s[:])
            nc.sync.dma_start(aps["dbg_uft"], uFT_sb[:])
            nc.sync.dma_start(aps["dbg_s2"], s2[:])
            nc.sync.dma_start(aps["dbg_b2"], b2[:])

        # ---- scores matmul, vocab-sharded. Evict 4 psum blocks into one wide
        # staging tile per out-DMA (HWDGE descriptor-gen bound otherwise);
        # evictions cycle 1:2 over DVE:Act (DVE is the busier engine).
        NBLK = 512
        GRP = 4
        with tc.tile_pool(name=f"psB{it}", bufs=5, space="PSUM") as psB, \
             tc.tile_pool(name=f"ost{it}", bufs=6) as ostage:
            k2 = 0
            for m in range(B // 128):
                n = 0
                while n < VSH // NBLK:
                    g = min(GRP, VSH // NBLK - n)
                    ot = ostage.tile([128, GRP * NBLK], bf16, tag="ot",
                                     name=f"ot{it}_{m}_{n}")
                    for i in range(g):
                        mm = psB.tile([128, NBLK], f32, tag="mm",
                                      name=f"mm{it}_{k2}")
                        nc.tensor.matmul(
                            mm[:], lhsT=lhsT[:, m * 128:(m + 1) * 128],
                            rhs=embT_s[:, (n + i) * NBLK:(n + i + 1) * NBLK],
                            start=True, stop=True)
                        dst = ot[:, i * NBLK:(i + 1) * NBLK]
                        # all evictions on Act: DVE must stay clear so the
                        # NEXT iteration's cumsum isn't queued behind them
                        nc.scalar.activation(dst, mm[:], Act.Copy)
                        k2 += 1
                    nc.sync.dma_start(
                        out[m * 128:(m + 1) * 128,
                            n * NBLK:(n + g) * NBLK], ot[:, 0:g * NBLK])
                    n += g


def _build(unroll=1, comm=None, num_devices=NCORES):
    if comm is None:
        comm = COMM_MODE
    import concourse.tile as tile
    from concourse import bacc, mybir

    f32 = mybir.dt.float32
    bf16 = mybir.dt.bfloat16
    i32 = mybir.dt.int32

    nc = bacc.Bacc("TRN2", target_bir_lowering=False, debug=False,
                   num_devices=num_devices, num_swdge_queues=2)
    aps = {
        "emb": nc.dram_tensor("emb", [N_ITEMS, D], f32, kind="ExternalInput").ap(),
        "embT": nc.dram_tensor("embT", [D, VSH], bf16, kind="ExternalInput").ap(),
        "offs": nc.dram_tensor("offs", [128, TH], i32, kind="ExternalInput").ap(),
        "rdiag": nc.dram_tensor("rdiag", [BSH, T * BSH], f32,
                                kind="ExternalInput").ap(),
        "pp": nc.dram_tensor("pp", [D, 2], f32, kind="ExternalInput").ap(),
        "i128": nc.dram_tensor("i128", [128, 128], f32, kind="ExternalInput").ap(),
        "out": nc.dram_tensor("out", [B, VSH], bf16, kind="ExternalOutput").ap(),
        "perm": nc.dram_tensor("perm", [1, NCORES], bf16,
                               kind="ExternalOutput").ap(),
    }
    if DEBUG_DUMP:
        aps["dbg_h"] = nc.dram_tensor("dbg_h", [128, T * BSH], f32,
                                      kind="ExternalOutput").ap()
        aps["dbg_spk"] = nc.dram_tensor("dbg_spk", [128, T * BSH], f32,
                                        kind="ExternalOutput").ap()
        aps["dbg_lhsT"] = nc.dram_tensor("dbg_lhsT", [D, B], bf16,
                                         kind="ExternalOutput").ap()
        aps["dbg_uo"] = nc.dram_tensor("dbg_uo", [128, WSLOT], f32,
                                       kind="ExternalOutput").ap()
        aps["dbg_uft"] = nc.dram_tensor("dbg_uft", [128, T * BSH], f32,
                                        kind="ExternalOutput").ap()
        aps["dbg_s2"] = nc.dram_tensor("dbg_s2", [D, T], f32,
                                       kind="ExternalOutput").ap()
        aps["dbg_b2"] = nc.dram_tensor("dbg_b2", [D, T], f32,
                                       kind="ExternalOutput").ap()
    cs = None
    if comm in ("rdma", "rdma_nostats"):
        groups = [list(range(num_devices))]
        nc._bir_kernel_barrier_sem_replica_groups.extend(set(g) for g in groups)
        cs = _Comm(nc)
    u32 = mybir.dt.uint32
    with tile.TileContext(nc) as tc:
        with tc.tile_pool(name="commp", bufs=1) as commp:
            recvs = [commp.tile([128, NCORES * WSLOT],
                                mybir.dt.bfloat16, name=f"recv{p}")
                     for p in range(2)]
            cons = {
                "offs_s": commp.tile([128, TH], i32, name="c_offs"),
                "rdiag_s": commp.tile([B // NCORES, T * B // NCORES], f32,
                                      name="c_rdiag"),
                "pp_s": commp.tile([D, 2], f32, name="c_pp"),
                "i128_s": commp.tile([128, 128], f32, name="c_i128"),
                "pid_s": commp.tile([1, 1], u32, name="c_pid"),
                "eps_t": commp.tile([D, 1], f32, name="c_eps"),
                "embT_s": commp.tile([D, VSH], bf16, name="c_embT"),
            }
            nc.sync.dma_start(cons["offs_s"][:], aps["offs"])
            nc.sync.dma_start(cons["rdiag_s"][:], aps["rdiag"])
            nc.sync.dma_start(cons["pp_s"][:], aps["pp"])
            nc.sync.dma_start(cons["i128_s"][:], aps["i128"])
            nc.sync.dma_start(cons["pid_s"][:],
                              nc.partition_id_tensor[0:1, 0:1])
            nc.vector.memset(cons["eps_t"][:], 1e-5)
            for q in range(4):
                nc.scalar.dma_start(
                    cons["embT_s"][:, q * (VSH // 4):(q + 1) * (VSH // 4)],
                    aps["embT"][:, q * (VSH // 4):(q + 1) * (VSH // 4)])
            for it in range(unroll):
                _emit_iteration(nc, tc, aps, cs, recvs[it % 2], cons, it=it,
                                comm=comm)
    if cs is not None and cs.post_waits:
        _attach_post_waits(nc, cs.post_waits)
    nc.compile()
    return nc


def _prep_inputs(seq, lengths, emb_table, gamma, beta):
    seq = np.asarray(seq)
    lengths = np.asarray(lengths)
    emb_table = np.asarray(emb_table, dtype=np.float32)
    gamma = np.asarray(gamma, dtype=np.float32)
    beta = np.asarray(beta, dtype=np.float32)

    emb_full = emb_table.copy()
    emb_full[0, :] = 0.0

    tt = np.arange(1, T + 1, dtype=np.float64)[None, :]
    denom = np.minimum(tt, lengths.astype(np.float64)[:, None])
    rd = (1.0 / denom).astype(np.float32)                      # [B, T]

    embT_full = np.zeros((D, NCORES * VSH), dtype=ml_dtypes.bfloat16)
    embT_full[:, :N_ITEMS] = emb_full.T.astype(ml_dtypes.bfloat16)

    pp = np.stack([gamma, beta], axis=1).astype(np.float32)    # [128, 2]
    i128 = np.eye(128, dtype=np.float32)

    in_maps = []
    for c in range(NCORES):
        sl = slice(c * BSH, (c + 1) * BSH)
        seq_c = seq[sl].astype(np.int32)                       # [64, 50]
        offs_c = np.concatenate([seq_c[:, 0::2], seq_c[:, 1::2]], axis=0)
        offs_c = np.ascontiguousarray(offs_c)                  # [128, 25]
        rd_c = rd[sl]                                          # [64, 50]
        r3 = np.zeros((BSH, T, BSH), dtype=np.float32)
        for b in range(BSH):
            r3[b, :, b] = rd_c[b]
        rdiag_c = np.ascontiguousarray(r3.reshape(BSH, T * BSH))
        embT_c = np.ascontiguousarray(embT_full[:, c * VSH:(c + 1) * VSH])
        in_maps.append({
            "emb": emb_full, "embT": embT_c, "offs": offs_c,
            "rdiag": rdiag_c, "pp": pp, "i128": i128,
        })
    return in_maps


def _assemble(results, use_perm=True):
    """Reorder each core's output rows using its slot->sender tag row, then
    concatenate vocab shards."""
    scores = np.empty((B, NCORES * VSH), dtype=np.float32)
    for c in range(NCORES):
        if use_perm:
            perm = np.asarray(results[c]["perm"][0], dtype=np.float32).astype(int)
            assert sorted(perm.tolist()) == list(range(NCORES)), (
                f"core {c}: bad uo-exchange tags {perm}")
        else:
            perm = np.arange(NCORES)
        oc = np.asarray(results[c]["out"], dtype=np.float32)
        dst = scores[:, c * VSH:(c + 1) * VSH]
        for k in range(NCORES):
            dst[perm[k] * BSH:(perm[k] + 1) * BSH] = oc[k * BSH:(k + 1) * BSH]
    return np.ascontiguousarray(scores[:, :N_ITEMS])


def _cached_runner(nc, reps_key):
    """Build (once) a jitted shard_map runner with device-resident input
    placement for repeated timed executions of nc's single bass_exec."""
    import jax
    from jax.sharding import Mesh, PartitionSpec
    from jax.experimental.shard_map import shard_map
    from concourse import mybir
    from concourse.bass2jax import (_bass_exec_p, partition_id_tensor,
                                    install_neuronx_cc_hook)
    install_neuronx_cc_hook()

    in_names, out_names, out_avals = [], [], []
    for alloc in nc.m.functions[0].allocations:
        if not isinstance(alloc, mybir.MemoryLocationSet):
            continue
        name = alloc.memorylocations[0].name
        if alloc.kind == "ExternalInput":
            if nc.partition_id_tensor is None or name != nc.partition_id_tensor.name:
                in_names.append(name)
        elif alloc.kind == "ExternalOutput":
            out_names.append(name)
            out_avals.append(jax.core.ShapedArray(
                tuple(alloc.tensor_shape), mybir.dt.np(alloc.dtype)))
    n_params = len(in_names)
    all_in = list(in_names) + list(out_names)
    if nc.partition_id_tensor is not None:
        all_in.append(nc.partition_id_tensor.name)

    def _body(*args):
        operands = list(args)
        if nc.partition_id_tensor is not None:
            operands.append(partition_id_tensor())
        return tuple(_bass_exec_p.bind(
            *operands, out_avals=tuple(out_avals), in_names=tuple(all_in),
            out_names=tuple(out_names), lowering_input_output_aliases=(),
            sim_require_finite=True, sim_require_nnan=True, nc=nc))

    mesh = Mesh(np.asarray(jax.devices()[:NCORES]), ("core",))
    n_outs = len(out_names)
    f = jax.jit(shard_map(
        _body, mesh=mesh,
        in_specs=(PartitionSpec("core"),) * (n_params + n_outs),
        out_specs=(PartitionSpec("core"),) * n_outs, check_rep=False))
    return f, in_names, out_avals


def benchmark(seq, lengths, emb_table, gamma, beta, unroll=16, pairs=30):
    """Estimate per-iteration device time via the slope between a 1x and a
    Kx-unrolled build of the same program (identical I/O staging costs).
    Executions are interleaved in (1x, Kx) pairs so axon-terminal drift
    cancels; the median pair-difference / (K-1) is the per-iteration time."""
    import jax, time, statistics
    in_maps = _prep_inputs(seq, lengths, emb_table, gamma, beta)
    if "nc" not in _CACHE:
        _CACHE["nc"] = _build()
    key = f"nc{unroll}"
    if key not in _CACHE:
        _CACHE[key] = _build(unroll=unroll)

    runners = []
    for nc in (_CACHE["nc"], _CACHE[key]):
        f, in_names, out_avals = _cached_runner(nc, None)
        per_core = [[np.asarray(m[nm]) for nm in in_names] for m in in_maps]
        ci = [jax.device_put(np.concatenate(
            [per_core[c][i] for c in range(NCORES)], axis=0))
            for i in range(len(in_names))]
        cz = [jax.device_put(np.zeros((NCORES * a.shape[0], *a.shape[1:]),
                                      a.dtype)) for a in out_avals]
        outx = f(*ci, *cz)
        jax.block_until_ready(outx)
        runners.append((f, ci, cz))

    def run_one(i):
        f, ci, cz = runners[i]
        t0 = time.perf_counter()
        outx = f(*ci, *cz)
        jax.block_until_ready(outx)
        return time.perf_counter() - t0

    diffs = []
    for _ in range(pairs):
        a = run_one(0)
        b = run_one(1)
        diffs.append(b - a)
    diffs.sort()
    med = diffs[len(diffs) // 2]
    per_iter_ns = med / (unroll - 1) * 1e9
    return per_iter_ns, {
        "median_diff_ms": med * 1e3,
        "mean_diff_ms": statistics.mean(diffs) * 1e3,
        "stdev_ms": statistics.stdev(diffs) * 1e3,
        "unroll": unroll, "pairs": pairs,
    }


def kernel(seq, lengths, emb_table, gamma, beta, trace=False):
    global LAST_EXEC_NS, LAST_RESULTS
    from concourse.bass_utils import run_bass_kernel_spmd

    if "nc" not in _CACHE:
        _CACHE["nc"] = _build()
    nc = _CACHE["nc"]

    in_maps = _prep_inputs(seq, lengths, emb_table, gamma, beta)
    res = run_bass_kernel_spmd(nc, in_maps, core_ids=list(range(NCORES)))
    LAST_EXEC_NS = res.exec_time_ns
    LAST_RESULTS = res
    return _assemble(res.results, use_perm=(COMM_MODE == "rdma"))


# revision 31
# speedup vs baseline: 1.2489x; 1.2489x over previous
"""Trainium2 Bass kernel for nn_BPRMF (segment_reduce): gather -> running-mean
-> BatchNorm(train) -> LIF spiking recurrence -> scores matmul.

Sharding over 8 NeuronCores:
  - gather/cumsum/BN/LIF: data-parallel over batch (64 rows/core); BN batch
    stats via AllReduce; LIF output exchanged via direct core-to-core
    remote-DMA broadcasts (SBUF->peer SBUF, no DRAM round trip).
  - scores matmul + output: vocab-sharded (12800 item columns/core).

Self-contained: hardcodes shapes, builds/compiles the Bass program on first
call, caches it for the process lifetime.
"""
import sys

sys.path.insert(0, "/opt/trn_rl_repo")

import numpy as np
import ml_dtypes

N_ITEMS = 100001
D = 128
T = 50
B = 512
NCORES = 8
BSH = B // NCORES          # 64 batch rows per core
VSH = 12800                # vocab shard per core (8*12800 = 102400 >= 100001)
TH = T // 2                # 25: gather packs two time-halves on 128 partitions
TAU = 2.0
V_TH = 1.0
BN_EPS = 1e-5
TSPLIT = 24                # stats A/B split (multiple of 8)
A_LAUNCH_J = 16            # gather step after which AR-A is launched on Pool
WSLOT = BSH + 1            # 65: uo payload = 64 data cols + 1 tag col

_CACHE = {}
LAST_EXEC_NS = None
LAST_RESULTS = None
DEBUG_DUMP = False
COMM_MODE = "cc"   # "cc": collective AllGather exchange; "rdma": direct peer-SBUF


class _Comm:
    """Cross-core exchange state for one build (sems + deferred waits)."""

    def __init__(self, nc):
        self.rsem = nc.alloc_semaphore(name="uo_rsem")
        self.lsem = nc.alloc_semaphore(name="uo_lsem")
        # (engine, target_inst_name, sem, value) to insert post-scheduling
        self.post_waits = []
        self.barrier_sem = nc._bir_kernel_barrier_sem
        self.barrier_val = nc.bir_kernel_barrier_sem_inc


def _attach_post_waits(nc, post_waits):
    """Insert standalone engine sem-wait instructions directly before their
    target instructions, after Tile scheduling (whose simulator cannot model
    remotely-incremented semaphores)."""
    fn = nc.m.functions[0]
    for eng, target, sem, val in post_waits:
        w = eng.wait_ge(sem, val)
        wname = w.ins.name
        wobj = None
        for blk in fn.blocks:
            insts = blk.instructions
            names = [i.name for i in insts]
            if wname in names:
                wobj = insts[names.index(wname)]
                blk.instructions = [i for i in insts if i.name != wname]
                break
        assert wobj is not None, f"wait {wname} not found"
        placed = False
        for blk in fn.blocks:
            insts = blk.instructions
            names = [i.name for i in insts]
            if target in names:
                insts.insert(names.index(target), wobj)
                blk.instructions = insts
                placed = True
                break
        assert placed, f"target {target} not found for wait insertion"


def _emit_iteration(nc, tc, aps, cs, recv, cons, it=0, comm="rdma"):
    """Emit one full pipeline iteration. Pools are scoped to the call so an
    unrolled timing build reuses the same on-chip space serially."""
    import concourse.bass as bass
    from concourse import mybir
    from contextlib import ExitStack

    f32 = mybir.dt.float32
    bf16 = mybir.dt.bfloat16
    i32 = mybir.dt.int32
    u32 = mybir.dt.uint32
    Alu = mybir.AluOpType
    Act = mybir.ActivationFunctionType

    emb, embT, offs, rdiag, pp, i128, out, perm = (
        aps["emb"], aps["embT"], aps["offs"], aps["rdiag"], aps["pp"],
        aps["i128"], aps["out"], aps["perm"])
    groups = [list(range(NCORES))]
    NA, NB_ = TSPLIT, T - TSPLIT

    # loop-invariant tiles, loaded once before iteration 0 (reloading them
    # per iteration adds WAR edges onto the previous iteration's consumers,
    # serializing the unrolled pipeline)
    offs_s, rdiag_s, pp_s, i128_s, pid_s, eps_t, embT_s = (
        cons["offs_s"], cons["rdiag_s"], cons["pp_s"], cons["i128_s"],
        cons["pid_s"], cons["eps_t"], cons["embT_s"])

    with ExitStack() as ctx:
        con = ctx.enter_context(tc.tile_pool(name=f"con{it}", bufs=1))
        work = ctx.enter_context(tc.tile_pool(name=f"work{it}", bufs=1))
        dr = ctx.enter_context(tc.tile_pool(name=f"dr{it}", bufs=1, space="DRAM"))

        # DRAM staging for the two stats AllReduces
        cc_inA = dr.tile([D, 2 * NA], f32, name=f"cc_inA{it}")
        cc_outA = dr.tile([D, 2 * NA], f32, addr_space="Shared",
                          name=f"cc_outA{it}")
        cc_inB = dr.tile([D, 2 * NB_], f32, name=f"cc_inB{it}")
        cc_outB = dr.tile([D, 2 * NB_], f32, addr_space="Shared",
                          name=f"cc_outB{it}")

        # ---- gather: offs column j holds (t=2j) on partitions p<64 and
        # (t=2j+1) on p>=64, so gather j completes ALL data for t<=2j+1. The
        # upper 64 partitions (odd t) are copied down to G2 in chunks
        # pipelined with the gather stream (tensor ops need operands at the
        # same start partition).
        G = con.tile([128, TH * D], f32, name=f"G{it}")
        G2 = con.tile([BSH, TH * D], f32, name=f"G2{it}")
        GCH = 2

        # cumsum / transpose / stats state
        pf0 = work.tile([BSH, D], f32, name=f"pf0{it}")
        pf1 = work.tile([BSH, D], f32, name=f"pf1{it}")
        pfs = [pf0, pf1]
        packed = work.tile([D, 2 * NA], f32, name=f"packed{it}")
        packedB = work.tile([D, 2 * NB_], f32, name=f"packedB{it}")
        sqbs = [work.tile([128, 8 * BSH], f32, name=f"sqb{it}_{p}")
                for p in range(2)]

        # uFT streams through 2 rotating PSUM banks into an SBUF copy: keeping
        # it fully PSUM-resident (7 banks) serializes consecutive iterations
        # against the scores matmuls' banks. Layout per iteration: uftp banks
        # 0-1, acc bank 2, psB (scores) banks 3-7 -- so iteration N+1's
        # transposes never wait on iteration N's scores.
        uftp = ctx.enter_context(tc.tile_pool(name=f"uftp{it}", bufs=2,
                                              space="PSUM"))
        accp = ctx.enter_context(tc.tile_pool(name=f"accp{it}", bufs=1,
                                              space="PSUM"))
        acc = accp.tile([128, BSH], f32, name=f"acc{it}")
        uFT_sb = con.tile([128, T * BSH], f32, name=f"uFTsb{it}")
        cur_chunk = [None]

        TPB = 8

        def stats_chunk(t0, t1):
            n = t1 - t0
            xs = cur_chunk[0][:, 0:n * BSH]
            # evict this chunk to SBUF for the later h/LIF reads
            nc.scalar.activation(uFT_sb[:, t0 * BSH:t1 * BSH], xs, Act.Copy)
            if t1 <= TSPLIT:
                dst_s = packed[:, t0:t1]
                dst_q = packed[:, NA + t0:NA + t1]
                ccdst_s = cc_inA[:, t0:t1]
                ccdst_q = cc_inA[:, NA + t0:NA + t1]
            else:
                dst_s = packedB[:, t0 - TSPLIT:t1 - TSPLIT]
                dst_q = packedB[:, NB_ + t0 - TSPLIT:NB_ + t1 - TSPLIT]
                ccdst_s = cc_inB[:, t0 - TSPLIT:t1 - TSPLIT]
                ccdst_q = cc_inB[:, NB_ + t0 - TSPLIT:NB_ + t1 - TSPLIT]
            sqb = sqbs[(t0 // TPB) % 2]
            nc.vector.tensor_reduce(
                out=dst_s, in_=xs.rearrange("p (t b) -> p t b", t=n),
                axis=mybir.AxisListType.X, op=Alu.add)
            nc.scalar.activation(sqb[:, 0:n * BSH], xs, Act.Square)
            nc.vector.tensor_reduce(
                out=dst_q,
                in_=sqb[:, 0:n * BSH].rearrange("p (t b) -> p t b", t=n),
                axis=mybir.AxisListType.X, op=Alu.add)
            # stage incrementally so the AllReduce's input is ready in DRAM
            # the moment the collective launches
            nc.sync.dma_start(ccdst_s, dst_s)
            nc.sync.dma_start(ccdst_q, dst_q)

        def emit_cumsum_steps(j):
            for t in (2 * j, 2 * j + 1):
                if t >= T:
                    continue
                if t % TPB == 0:
                    cur_chunk[0] = uftp.tile([128, TPB * BSH], f32,
                                             tag="uftc", name=f"uftc{it}_{t}")
                src = (G[0:BSH, (t // 2) * D:(t // 2 + 1) * D] if t % 2 == 0
                       else G2[0:BSH, (t // 2) * D:(t // 2 + 1) * D])
                pf = pfs[t % 2]
                if t == 0:
                    nc.vector.tensor_copy(pf[:], src)
                else:
                    nc.vector.tensor_tensor(out=pf[:], in0=pfs[(t - 1) % 2][:],
                                            in1=src, op=Alu.add)
                ti = t % TPB
                nc.tensor.matmul(cur_chunk[0][:, ti * BSH:(ti + 1) * BSH],
                                 lhsT=pf[:],
                                 rhs=rdiag_s[:, t * BSH:(t + 1) * BSH],
                                 start=True, stop=True)
                if (t + 1) % TPB == 0:
                    stats_chunk(t + 1 - TPB, t + 1)
                elif t == T - 1:
                    stats_chunk(T - T % TPB, T)

        # cumsum for a given j is only emitted once its G2 chunk's copy has
        # been emitted: Tile's dep tracking is emission-ordered, and a read
        # emitted before its writer gets no RAW edge (reads stale data).
        for j in range(TH):
            nc.gpsimd.indirect_dma_start(
                out=G[:, j * D:(j + 1) * D], out_offset=None, in_=emb,
                in_offset=bass.IndirectOffsetOnAxis(ap=offs_s[:, j:j + 1], axis=0),
            )
            if (j + 1) % GCH == 0:
                lo = (j + 1 - GCH) * D
                nc.sync.dma_start(G2[:, lo:(j + 1) * D], G[BSH:128, lo:(j + 1) * D])
                for jj in range(j + 1 - GCH, j + 1):
                    emit_cumsum_steps(jj)
            if j == A_LAUNCH_J and comm in ("rdma", "cc"):
                nc.gpsimd.collective_compute(
                    "AllReduce", Alu.add, replica_groups=groups,
                    ins=[cc_inA[:]], outs=[cc_outA[:]])
        if TH % GCH:
            lo = (TH - TH % GCH) * D
            nc.sync.dma_start(G2[:, lo:TH * D], G[BSH:128, lo:TH * D])
            for jj in range(TH - TH % GCH, TH):
                emit_cumsum_steps(jj)
        if comm in ("rdma", "cc"):
            nc.gpsimd.collective_compute(
                "AllReduce", Alu.add, replica_groups=groups,
                ins=[cc_inB[:]], outs=[cc_outB[:]])
        else:
            nc.sync.dma_start(cc_outA[:], cc_inA[:])
            nc.sync.dma_start(cc_outB[:], cc_inB[:])

        gstatsA = work.tile([D, 2 * NA], f32, name=f"gstatsA{it}")
        nc.sync.dma_start(gstatsA[:], cc_outA[:])
        gstatsB = work.tile([D, 2 * NB_], f32, name=f"gstatsB{it}")
        nc.sync.dma_start(gstatsB[:], cc_outB[:])

        # ---- BN affine params: h_t = x*s2_t + b2_t  (pre-divided by TAU)
        bh = work.tile([D, 1], f32, name=f"bh{it}")
        nc.vector.tensor_scalar(out=bh[:], in0=pp_s[:, 1:2],
                                scalar1=1.0 / TAU, scalar2=None, op0=Alu.mult)
        s2 = work.tile([D, T], f32, name=f"s2{it}")
        b2 = work.tile([D, T], f32, name=f"b2{it}")

        def emit_params(gst, n, col0, tag):
            mean = work.tile([D, n], f32, name=f"mean{tag}{it}")
            nc.vector.tensor_scalar(out=mean[:], in0=gst[:, 0:n],
                                    scalar1=1.0 / B, scalar2=None, op0=Alu.mult)
            ex2 = work.tile([D, n], f32, name=f"ex2{tag}{it}")
            nc.vector.tensor_scalar(out=ex2[:], in0=gst[:, n:2 * n],
                                    scalar1=1.0 / B, scalar2=None, op0=Alu.mult)
            var = work.tile([D, n], f32, name=f"var{tag}{it}")
            nc.vector.tensor_tensor(out=var[:], in0=mean[:], in1=mean[:],
                                    op=Alu.mult)
            nc.vector.tensor_tensor(out=var[:], in0=ex2[:], in1=var[:],
                                    op=Alu.subtract)
            std = work.tile([D, n], f32, name=f"std{tag}{it}")
            nc.scalar.activation(std[:], var[:], Act.Sqrt, bias=eps_t[:, 0:1])
            inv = work.tile([D, n], f32, name=f"inv{tag}{it}")
            nc.vector.reciprocal(inv[:], std[:])
            s2s = s2[:, col0:col0 + n]
            nc.vector.tensor_scalar(out=s2s, in0=inv[:], scalar1=pp_s[:, 0:1],
                                    scalar2=1.0 / TAU, op0=Alu.mult,
                                    op1=Alu.mult)
            ms = work.tile([D, n], f32, name=f"ms{tag}{it}")
            nc.vector.tensor_tensor(out=ms[:], in0=mean[:], in1=s2s,
                                    op=Alu.mult)
            nc.vector.scalar_tensor_tensor(
                out=b2[:, col0:col0 + n], in0=ms[:], scalar=-1.0,
                in1=bh[:, 0:1].to_broadcast((D, n)), op0=Alu.mult, op1=Alu.add)

        emit_params(gstatsA, NA, 0, "A")
        emit_params(gstatsB, NB_, TSPLIT, "B")

        # ---- LIF recurrence on the pre-reset voltage w:
        #   s_t = [w_t >= 1];  w_{t+1} = (w_t - s_t)/2 + h_{t+1}
        # h precomputed for every t into one big tile so the Act engine runs
        # ahead of the serial DVE chain; spikes accumulated on the idle PE
        # via identity-matmul PSUM accumulation (kills the tail reduce).
        h = con.tile([128, T * BSH], f32, name=f"h{it}")
        for t in range(T):
            nc.scalar.activation(h[:, t * BSH:(t + 1) * BSH],
                                 uFT_sb[:, t * BSH:(t + 1) * BSH],
                                 Act.Identity, scale=s2[:, t:t + 1],
                                 bias=b2[:, t:t + 1])

        w = work.tile([128, BSH], f32, name=f"w{it}")
        q = work.tile([128, BSH], f32, name=f"q{it}")
        spk = con.tile([128, T * BSH], f32, name=f"spk{it}")
        for t in range(T):
            ht = h[:, t * BSH:(t + 1) * BSH]
            if t == 0:
                nc.vector.tensor_copy(w[:], ht)
            else:
                nc.vector.scalar_tensor_tensor(
                    out=q[:], in0=w[:], scalar=1.0 / TAU, in1=ht,
                    op0=Alu.mult, op1=Alu.add)
                nc.vector.scalar_tensor_tensor(
                    out=w[:], in0=spk[:, (t - 1) * BSH:t * BSH],
                    scalar=-V_TH / TAU, in1=q[:], op0=Alu.mult, op1=Alu.add)
            nc.vector.tensor_scalar(out=spk[:, t * BSH:(t + 1) * BSH],
                                    in0=w[:], scalar1=V_TH, scalar2=None,
                                    op0=Alu.is_ge)
            nc.tensor.matmul(acc[:], lhsT=i128_s[:],
                             rhs=spk[:, t * BSH:(t + 1) * BSH],
                             start=(t == 0), stop=(t == T - 1))

        # uo payload [128, 65]: cols 0..63 = mean spikes (bf16), col 64 row 0
        # = this core's id tag (receivers recover the sender->slot map from it)
        uo = work.tile([128, WSLOT], bf16, name=f"uo{it}")
        uo_w = nc.scalar.activation(uo[:, 0:BSH], acc[:], Act.Identity,
                                    scale=1.0 / T)
        nc.vector.tensor_copy(uo[0:1, BSH:BSH + 1], pid_s[:])
        if comm in ("rdma", "rdma_nostats") and it > 0:
            cs.post_waits.append((nc.scalar, uo_w.ins.name, cs.lsem,
                                  16 * NCORES * it))

        # ---- uo exchange: 8 single-dest remote broadcasts (slot k -> peer
        # me^k), descriptors prepped on the idle Pool engine during the LIF,
        # fired by one trigger once uo is ready. Lands directly in peers'
        # SBUF recv tile -- no DRAM round trip, no collective.
        lhsT = work.tile([D, B], bf16, name=f"lhsT{it}")
        if comm == "cc":
            # collective AllGather fallback (v0-style): SBUF->DRAM->AG->SBUF
            ag_in = dr.tile([D, BSH], bf16, name=f"ag_in{it}")
            ag_out = dr.tile([NCORES * D, BSH], bf16, addr_space="Shared",
                             name=f"ag_out{it}")
            nc.sync.dma_start(ag_in[:], uo[:, 0:BSH])
            nc.gpsimd.collective_compute(
                "AllGather", Alu.bypass, replica_groups=groups,
                ins=[ag_in[:]], outs=[ag_out[:]])
            nc.sync.dma_start(
                lhsT[:].rearrange("p (c b) -> p c b", c=NCORES),
                ag_out[:].rearrange("(c p) b -> p c b", c=NCORES))
            for k in range(NCORES):
                nc.sync.dma_start(perm[0:1, k:k + 1], uo[0:1, BSH:BSH + 1])
        elif comm in ("rdma", "rdma_nostats"):
            for k in range(NCORES):
                rdests = [None] * NCORES
                rdests[k] = (0, k)
                nc.gpsimd.remote_dma_broadcast(
                    out_ap=recv[:, k * WSLOT:(k + 1) * WSLOT],
                    in_ap=uo[:, 0:WSLOT], remote_sem=cs.rsem,
                    local_sem=cs.lsem, rdests=rdests, queue_num=1)
            trig = nc.gpsimd.trigger_dma(count=None, queue_num=1)
            cs.post_waits.append((nc.gpsimd, trig.ins.name, cs.barrier_sem,
                                  cs.barrier_val))
            rthresh = 2 * NCORES * (it + 1)
            # assemble lhsT from the recv slots (data cols only); first copy
            # per engine carries the arrival wait
            for k in range(NCORES):
                src = recv[:, k * WSLOT:k * WSLOT + BSH]
                dst = lhsT[:, k * BSH:(k + 1) * BSH]
                if k % 2 == 0:
                    cp = nc.vector.tensor_copy(dst, src)
                    eng = nc.vector
                else:
                    cp = nc.scalar.activation(dst, src, Act.Copy)
                    eng = nc.scalar
                if k < 2:
                    cs.post_waits.append((eng, cp.ins.name, cs.rsem, rthresh))
            for k in range(NCORES):
                pdma = nc.sync.dma_start(
                    perm[0:1, k:k + 1],
                    recv[0:1, k * WSLOT + BSH:k * WSLOT + BSH + 1])
                if k == 0:
                    cs.post_waits.append((nc.sync, pdma.ins.name, cs.rsem,
                                          rthresh))
        else:
            # dev-only fallback (TimelineSim): replicate own uo into all slots
            for k in range(NCORES):
                nc.vector.tensor_copy(lhsT[:, k * BSH:(k + 1) * BSH],
                                      uo[:, 0:BSH])
                nc.sync.dma_start(perm[0:1, k:k + 1],
                                  uo[0:1, BSH:BSH + 1])

        if DEBUG_DUMP and it == 0:
            nc.sync.dma_start(aps["dbg_h"], h[:])
            nc.sync.dma_start(aps["dbg_spk"], spk[:])
            nc.sync.dma_start(aps["dbg_lhsT"], lhsT[:])
            dbg_uo_s = work.tile([128, WSLOT], mybir.dt.float32,
                                 name=f"dbg_uo_s{it}")
            nc.vector.tensor_copy(dbg_uo_s[:], uo[:])
            nc.sync.dma_start(aps["dbg_uo"], dbg_uo_s[:])
            nc.sync.dma_start(aps["dbg_uft"], uFT_sb[:])
            nc.sync.dma_start(aps["dbg_s2"], s2[:])
            nc.sync.dma_start(aps["dbg_b2"], b2[:])

        # ---- scores matmul, vocab-sharded. Evict 4 psum blocks into one wide
        # staging tile per out-DMA (HWDGE descriptor-gen bound otherwise);
        # evictions cycle 1:2 over DVE:Act (DVE is the busier engine).
        NBLK = 512
        GRP = 4
        with tc.tile_pool(name=f"psB{it}", bufs=5, space="PSUM") as psB, \
             tc.tile_pool(name=f"ost{it}", bufs=6) as ostage:
            k2 = 0
            for m in range(B // 128):
                n = 0
                while n < VSH // NBLK:
                    g = min(GRP, VSH // NBLK - n)
                    ot = ostage.tile([128, GRP * NBLK], bf16, tag="ot",
                                     name=f"ot{it}_{m}_{n}")
                    for i in range(g):
                        mm = psB.tile([128, NBLK], f32, tag="mm",
                                      name=f"mm{it}_{k2}")
                        nc.tensor.matmul(
                            mm[:], lhsT=lhsT[:, m * 128:(m + 1) * 128],
                            rhs=embT_s[:, (n + i) * NBLK:(n + i + 1) * NBLK],
                            start=True, stop=True)
                        dst = ot[:, i * NBLK:(i + 1) * NBLK]
                        # 1:2 DVE:Act eviction split -- keeps DVE light so the
                        # next iteration's cumsum isn't queued far behind
                        if k2 % 3 == 0:
                            nc.vector.tensor_copy(dst, mm[:])
                        else:
                            nc.scalar.activation(dst, mm[:], Act.Copy)
                        k2 += 1
                    nc.sync.dma_start(
                        out[m * 128:(m + 1) * 128,
                            n * NBLK:(n + g) * NBLK], ot[:, 0:g * NBLK])
                    n += g


def _build(unroll=1, comm=None, num_devices=NCORES):
    if comm is None:
        comm = COMM_MODE
    import concourse.tile as tile
    from concourse import bacc, mybir

    f32 = mybir.dt.float32
    bf16 = mybir.dt.bfloat16
    i32 = mybir.dt.int32

    nc = bacc.Bacc("TRN2", target_bir_lowering=False, debug=False,
                   num_devices=num_devices, num_swdge_queues=2)
    aps = {
        "emb": nc.dram_tensor("emb", [N_ITEMS, D], f32, kind="ExternalInput").ap(),
        "embT": nc.dram_tensor("embT", [D, VSH], bf16, kind="ExternalInput").ap(),
        "offs": nc.dram_tensor("offs", [128, TH], i32, kind="ExternalInput").ap(),
        "rdiag": nc.dram_tensor("rdiag", [BSH, T * BSH], f32,
                                kind="ExternalInput").ap(),
        "pp": nc.dram_tensor("pp", [D, 2], f32, kind="ExternalInput").ap(),
        "i128": nc.dram_tensor("i128", [128, 128], f32, kind="ExternalInput").ap(),
        "out": nc.dram_tensor("out", [B, VSH], bf16, kind="ExternalOutput").ap(),
        "perm": nc.dram_tensor("perm", [1, NCORES], bf16,
                               kind="ExternalOutput").ap(),
    }
    if DEBUG_DUMP:
        aps["dbg_h"] = nc.dram_tensor("dbg_h", [128, T * BSH], f32,
                                      kind="ExternalOutput").ap()
        aps["dbg_spk"] = nc.dram_tensor("dbg_spk", [128, T * BSH], f32,
                                        kind="ExternalOutput").ap()
        aps["dbg_lhsT"] = nc.dram_tensor("dbg_lhsT", [D, B], bf16,
                                         kind="ExternalOutput").ap()
        aps["dbg_uo"] = nc.dram_tensor("dbg_uo", [128, WSLOT], f32,
                                       kind="ExternalOutput").ap()
        aps["dbg_uft"] = nc.dram_tensor("dbg_uft", [128, T * BSH], f32,
                                        kind="ExternalOutput").ap()
        aps["dbg_s2"] = nc.dram_tensor("dbg_s2", [D, T], f32,
                                       kind="ExternalOutput").ap()
        aps["dbg_b2"] = nc.dram_tensor("dbg_b2", [D, T], f32,
                                       kind="ExternalOutput").ap()
    cs = None
    if comm in ("rdma", "rdma_nostats"):
        groups = [list(range(num_devices))]
        nc._bir_kernel_barrier_sem_replica_groups.extend(set(g) for g in groups)
        cs = _Comm(nc)
    u32 = mybir.dt.uint32
    with tile.TileContext(nc) as tc:
        with tc.tile_pool(name="commp", bufs=1) as commp:
            recvs = [commp.tile([128, NCORES * WSLOT],
                                mybir.dt.bfloat16, name=f"recv{p}")
                     for p in range(2)]
            cons = {
                "offs_s": commp.tile([128, TH], i32, name="c_offs"),
                "rdiag_s": commp.tile([B // NCORES, T * B // NCORES], f32,
                                      name="c_rdiag"),
                "pp_s": commp.tile([D, 2], f32, name="c_pp"),
                "i128_s": commp.tile([128, 128], f32, name="c_i128"),
                "pid_s": commp.tile([1, 1], u32, name="c_pid"),
                "eps_t": commp.tile([D, 1], f32, name="c_eps"),
                "embT_s": commp.tile([D, VSH], bf16, name="c_embT"),
            }
            nc.sync.dma_start(cons["offs_s"][:], aps["offs"])
            nc.sync.dma_start(cons["rdiag_s"][:], aps["rdiag"])
            nc.sync.dma_start(cons["pp_s"][:], aps["pp"])
            nc.sync.dma_start(cons["i128_s"][:], aps["i128"])
            nc.sync.dma_start(cons["pid_s"][:],
                              nc.partition_id_tensor[0:1, 0:1])
            nc.vector.memset(cons["eps_t"][:], 1e-5)
            for q in range(4):
                nc.scalar.dma_start(
                    cons["embT_s"][:, q * (VSH // 4):(q + 1) * (VSH // 4)],
                    aps["embT"][:, q * (VSH // 4):(q + 1) * (VSH // 4)])
            for it in range(unroll):
                _emit_iteration(nc, tc, aps, cs, recvs[it % 2], cons, it=it,
                                comm=comm)
    if cs is not None and cs.post_waits:
        _attach_post_waits(nc, cs.post_waits)
    nc.compile()
    return nc


def _prep_inputs(seq, lengths, emb_table, gamma, beta):
    seq = np.asarray(seq)
    lengths = np.asarray(lengths)
    emb_table = np.asarray(emb_table, dtype=np.float32)
    gamma = np.asarray(gamma, dtype=np.float32)
    beta = np.asarray(beta, dtype=np.float32)

    emb_full = emb_table.copy()
    emb_full[0, :] = 0.0

    tt = np.arange(1, T + 1, dtype=np.float64)[None, :]
    denom = np.minimum(tt, lengths.astype(np.float64)[:, None])
    rd = (1.0 / denom).astype(np.float32)                      # [B, T]

    embT_full = np.zeros((D, NCORES * VSH), dtype=ml_dtypes.bfloat16)
    embT_full[:, :N_ITEMS] = emb_full.T.astype(ml_dtypes.bfloat16)

    pp = np.stack([gamma, beta], axis=1).astype(np.float32)    # [128, 2]
    i128 = np.eye(128, dtype=np.float32)

    in_maps = []
    for c in range(NCORES):
        sl = slice(c * BSH, (c + 1) * BSH)
        seq_c = seq[sl].astype(np.int32)                       # [64, 50]
        offs_c = np.concatenate([seq_c[:, 0::2], seq_c[:, 1::2]], axis=0)
        offs_c = np.ascontiguousarray(offs_c)                  # [128, 25]
        rd_c = rd[sl]                                          # [64, 50]
        r3 = np.zeros((BSH, T, BSH), dtype=np.float32)
        for b in range(BSH):
            r3[b, :, b] = rd_c[b]
        rdiag_c = np.ascontiguousarray(r3.reshape(BSH, T * BSH))
        embT_c = np.ascontiguousarray(embT_full[:, c * VSH:(c + 1) * VSH])
        in_maps.append({
            "emb": emb_full, "embT": embT_c, "offs": offs_c,
            "rdiag": rdiag_c, "pp": pp, "i128": i128,
        })
    return in_maps


def _assemble(results, use_perm=True):
    """Reorder each core's output rows using its slot->sender tag row, then
    concatenate vocab shards."""
    scores = np.empty((B, NCORES * VSH), dtype=np.float32)
    for c in range(NCORES):
        if use_perm:
            perm = np.asarray(results[c]["perm"][0], dtype=np.float32).astype(int)
            assert sorted(perm.tolist()) == list(range(NCORES)), (
                f"core {c}: bad uo-exchange tags {perm}")
        else:
            perm = np.arange(NCORES)
        oc = np.asarray(results[c]["out"], dtype=np.float32)
        dst = scores[:, c * VSH:(c + 1) * VSH]
        for k in range(NCORES):
            dst[perm[k] * BSH:(perm[k] + 1) * BSH] = oc[k * BSH:(k + 1) * BSH]
    return np.ascontiguousarray(scores[:, :N_ITEMS])


def _cached_runner(nc, reps_key):
    """Build (once) a jitted shard_map runner with device-resident input
    placement for repeated timed executions of nc's single bass_exec."""
    import jax
    from jax.sharding import Mesh, PartitionSpec
    from jax.experimental.shard_map import shard_map
    from concourse import mybir
    from concourse.bass2jax import (_bass_exec_p, partition_id_tensor,
                                    install_neuronx_cc_hook)
    install_neuronx_cc_hook()

    in_names, out_names, out_avals = [], [], []
    for alloc in nc.m.functions[0].allocations:
        if not isinstance(alloc, mybir.MemoryLocationSet):
            continue
        name = alloc.memorylocations[0].name
        if alloc.kind == "ExternalInput":
            if nc.partition_id_tensor is None or name != nc.partition_id_tensor.name:
                in_names.append(name)
        elif alloc.kind == "ExternalOutput":
            out_names.append(name)
            out_avals.append(jax.core.ShapedArray(
                tuple(alloc.tensor_shape), mybir.dt.np(alloc.dtype)))
    n_params = len(in_names)
    all_in = list(in_names) + list(out_names)
    if nc.partition_id_tensor is not None:
        all_in.append(nc.partition_id_tensor.name)

    def _body(*args):
        operands = list(args)
        if nc.partition_id_tensor is not None:
            operands.append(partition_id_tensor())
        return tuple(_bass_exec_p.bind(
            *operands, out_avals=tuple(out_avals), in_names=tuple(all_in),
            out_names=tuple(out_names), lowering_input_output_aliases=(),
            sim_require_finite=True, sim_require_nnan=True, nc=nc))

    mesh = Mesh(np.asarray(jax.devices()[:NCORES]), ("core",))
    n_outs = len(out_names)
    f = jax.jit(shard_map(
        _body, mesh=mesh,
        in_specs=(PartitionSpec("core"),) * (n_params + n_outs),
        out_specs=(PartitionSpec("core"),) * n_outs, check_rep=False))
    return f, in_names, out_avals


def benchmark(seq, lengths, emb_table, gamma, beta, unroll=16, pairs=30):
    """Estimate per-iteration device time via the slope between a 1x and a
    Kx-unrolled build of the same program (identical I/O staging costs).
    Executions are interleaved in (1x, Kx) pairs so axon-terminal drift
    cancels; the median pair-difference / (K-1) is the per-iteration time."""
    import jax, time, statistics
    in_maps = _prep_inputs(seq, lengths, emb_table, gamma, beta)
    if "nc" not in _CACHE:
        _CACHE["nc"] = _build()
    key = f"nc{unroll}"
    if key not in _CACHE:
        _CACHE[key] = _build(unroll=unroll)

    runners = []
    for nc in (_CACHE["nc"], _CACHE[key]):
        f, in_names, out_avals = _cached_runner(nc, None)
        per_core = [[np.asarray(m[nm]) for nm in in_names] for m in in_maps]
        ci = [jax.device_put(np.concatenate(
            [per_core[c][i] for c in range(NCORES)], axis=0))
            for i in range(len(in_names))]
        cz = [jax.device_put(np.zeros((NCORES * a.shape[0], *a.shape[1:]),
                                      a.dtype)) for a in out_avals]
        outx = f(*ci, *cz)
        jax.block_until_ready(outx)
        runners.append((f, ci, cz))

    def run_one(i):
        f, ci, cz = runners[i]
        t0 = time.perf_counter()
        outx = f(*ci, *cz)
        jax.block_until_ready(outx)
        return time.perf_counter() - t0

    diffs = []
    for _ in range(pairs):
        a = run_one(0)
        b = run_one(1)
        diffs.append(b - a)
    diffs.sort()
    med = diffs[len(diffs) // 2]
    per_iter_ns = med / (unroll - 1) * 1e9
    return per_iter_ns, {
        "median_diff_ms": med * 1e3,
        "mean_diff_ms": statistics.mean(diffs) * 1e3,
        "stdev_ms": statistics.stdev(diffs) * 1e3,
        "unroll": unroll, "pairs": pairs,
    }


def kernel(seq, lengths, emb_table, gamma, beta, trace=False):
    global LAST_EXEC_NS, LAST_RESULTS
    from concourse.bass_utils import run_bass_kernel_spmd

    if "nc" not in _CACHE:
        _CACHE["nc"] = _build()
    nc = _CACHE["nc"]

    in_maps = _prep_inputs(seq, lengths, emb_table, gamma, beta)
    res = run_bass_kernel_spmd(nc, in_maps, core_ids=list(range(NCORES)))
    LAST_EXEC_NS = res.exec_time_ns
    LAST_RESULTS = res
    return _assemble(res.results, use_perm=(COMM_MODE == "rdma"))


# revision 35
# speedup vs baseline: 1.3721x; 1.0986x over previous
"""Trainium2 Bass kernel for nn_BPRMF (segment_reduce): gather -> running-mean
-> BatchNorm(train) -> LIF spiking recurrence -> scores matmul.

Sharding over 8 NeuronCores:
  - gather/cumsum/BN/LIF: data-parallel over batch (64 rows/core); BN batch
    stats via AllReduce; LIF output exchanged via direct core-to-core
    remote-DMA broadcasts (SBUF->peer SBUF, no DRAM round trip).
  - scores matmul + output: vocab-sharded (12800 item columns/core).

Self-contained: hardcodes shapes, builds/compiles the Bass program on first
call, caches it for the process lifetime.
"""
import sys

sys.path.insert(0, "/opt/trn_rl_repo")

import numpy as np
import ml_dtypes

N_ITEMS = 100001
D = 128
T = 50
B = 512
NCORES = 8
BSH = B // NCORES          # 64 batch rows per core
VSH = 12800                # vocab shard per core (8*12800 = 102400 >= 100001)
TH = T // 2                # 25: gather packs two time-halves on 128 partitions
TAU = 2.0
V_TH = 1.0
BN_EPS = 1e-5
TSPLIT = 24                # stats A/B split (multiple of 8)
A_LAUNCH_J = 16            # gather step after which AR-A is launched on Pool
WSLOT = BSH + 1            # 65: uo payload = 64 data cols + 1 tag col

_CACHE = {}
LAST_EXEC_NS = None
LAST_RESULTS = None
DEBUG_DUMP = False
COMM_MODE = "cc"   # "cc": collective AllGather exchange; "rdma": direct peer-SBUF


class _Comm:
    """Cross-core exchange state for one build (sems + deferred waits)."""

    def __init__(self, nc):
        self.rsem = nc.alloc_semaphore(name="uo_rsem")
        self.lsem = nc.alloc_semaphore(name="uo_lsem")
        # (engine, target_inst_name, sem, value) to insert post-scheduling
        self.post_waits = []
        self.barrier_sem = nc._bir_kernel_barrier_sem
        self.barrier_val = nc.bir_kernel_barrier_sem_inc


def _attach_post_waits(nc, post_waits):
    """Insert standalone engine sem-wait instructions directly before their
    target instructions, after Tile scheduling (whose simulator cannot model
    remotely-incremented semaphores)."""
    fn = nc.m.functions[0]
    for eng, target, sem, val in post_waits:
        w = eng.wait_ge(sem, val)
        wname = w.ins.name
        wobj = None
        for blk in fn.blocks:
            insts = blk.instructions
            names = [i.name for i in insts]
            if wname in names:
                wobj = insts[names.index(wname)]
                blk.instructions = [i for i in insts if i.name != wname]
                break
        assert wobj is not None, f"wait {wname} not found"
        placed = False
        for blk in fn.blocks:
            insts = blk.instructions
            names = [i.name for i in insts]
            if target in names:
                insts.insert(names.index(target), wobj)
                blk.instructions = insts
                placed = True
                break
        assert placed, f"target {target} not found for wait insertion"


def _emit_iteration(nc, tc, aps, cs, recv, cons, it=0, comm="rdma"):
    """Emit one full pipeline iteration. Pools are scoped to the call so an
    unrolled timing build reuses the same on-chip space serially."""
    import concourse.bass as bass
    from concourse import mybir
    from contextlib import ExitStack

    f32 = mybir.dt.float32
    bf16 = mybir.dt.bfloat16
    i32 = mybir.dt.int32
    u32 = mybir.dt.uint32
    Alu = mybir.AluOpType
    Act = mybir.ActivationFunctionType

    emb, embT, offs, rdiag, pp, i128, out, perm = (
        aps["emb"], aps["embT"], aps["offs"], aps["rdiag"], aps["pp"],
        aps["i128"], aps["out"], aps["perm"])
    groups = [list(range(NCORES))]
    NA, NB_ = TSPLIT, T - TSPLIT

    # loop-invariant tiles, loaded once before iteration 0 (reloading them
    # per iteration adds WAR edges onto the previous iteration's consumers,
    # serializing the unrolled pipeline)
    offs_s, rdiag_s, pp_s, i128_s, pid_s, eps_t, embT_s = (
        cons["offs_s"], cons["rdiag_s"], cons["pp_s"], cons["i128_s"],
        cons["pid_s"], cons["eps_t"], cons["embT_s"])

    with ExitStack() as ctx:
        con = ctx.enter_context(tc.tile_pool(name=f"con{it}", bufs=1))
        work = ctx.enter_context(tc.tile_pool(name=f"work{it}", bufs=1))
        dr = ctx.enter_context(tc.tile_pool(name=f"dr{it}", bufs=1, space="DRAM"))

        # DRAM staging for the two stats AllReduces
        cc_inA = dr.tile([D, 2 * NA], f32, name=f"cc_inA{it}")
        cc_outA = dr.tile([D, 2 * NA], f32, addr_space="Shared",
                          name=f"cc_outA{it}")
        cc_inB = dr.tile([D, 2 * NB_], f32, name=f"cc_inB{it}")
        cc_outB = dr.tile([D, 2 * NB_], f32, addr_space="Shared",
                          name=f"cc_outB{it}")

        # ---- gather: offs column j holds (t=2j) on partitions p<64 and
        # (t=2j+1) on p>=64, so gather j completes ALL data for t<=2j+1. The
        # upper 64 partitions (odd t) are copied down to G2 in chunks
        # pipelined with the gather stream (tensor ops need operands at the
        # same start partition).
        G = con.tile([128, TH * D], f32, name=f"G{it}")
        G2 = con.tile([BSH, TH * D], f32, name=f"G2{it}")
        GCH = 2

        # cumsum / transpose / stats state
        pf0 = work.tile([BSH, D], f32, name=f"pf0{it}")
        pf1 = work.tile([BSH, D], f32, name=f"pf1{it}")
        pfs = [pf0, pf1]
        packed = work.tile([D, 2 * NA], f32, name=f"packed{it}")
        packedB = work.tile([D, 2 * NB_], f32, name=f"packedB{it}")
        sqbs = [work.tile([128, 8 * BSH], f32, name=f"sqb{it}_{p}")
                for p in range(2)]

        # uFT streams through 2 rotating PSUM banks into an SBUF copy: keeping
        # it fully PSUM-resident (7 banks) serializes consecutive iterations
        # against the scores matmuls' banks. Layout per iteration: uftp banks
        # 0-1, acc bank 2, psB (scores) banks 3-7 -- so iteration N+1's
        # transposes never wait on iteration N's scores.
        uftp = ctx.enter_context(tc.tile_pool(name=f"uftp{it}", bufs=2,
                                              space="PSUM"))
        accp = ctx.enter_context(tc.tile_pool(name=f"accp{it}", bufs=1,
                                              space="PSUM"))
        acc = accp.tile([128, BSH], f32, name=f"acc{it}")
        uFT_sb = con.tile([128, T * BSH], f32, name=f"uFTsb{it}")
        cur_chunk = [None]

        TPB = 8

        def stats_chunk(t0, t1):
            n = t1 - t0
            xs = cur_chunk[0][:, 0:n * BSH]
            # evict this chunk to SBUF for the later h/LIF reads
            nc.scalar.activation(uFT_sb[:, t0 * BSH:t1 * BSH], xs, Act.Copy)
            if t1 <= TSPLIT:
                dst_s = packed[:, t0:t1]
                dst_q = packed[:, NA + t0:NA + t1]
                ccdst_s = cc_inA[:, t0:t1]
                ccdst_q = cc_inA[:, NA + t0:NA + t1]
            else:
                dst_s = packedB[:, t0 - TSPLIT:t1 - TSPLIT]
                dst_q = packedB[:, NB_ + t0 - TSPLIT:NB_ + t1 - TSPLIT]
                ccdst_s = cc_inB[:, t0 - TSPLIT:t1 - TSPLIT]
                ccdst_q = cc_inB[:, NB_ + t0 - TSPLIT:NB_ + t1 - TSPLIT]
            sqb = sqbs[(t0 // TPB) % 2]
            nc.vector.tensor_reduce(
                out=dst_s, in_=xs.rearrange("p (t b) -> p t b", t=n),
                axis=mybir.AxisListType.X, op=Alu.add)
            nc.scalar.activation(sqb[:, 0:n * BSH], xs, Act.Square)
            nc.vector.tensor_reduce(
                out=dst_q,
                in_=sqb[:, 0:n * BSH].rearrange("p (t b) -> p t b", t=n),
                axis=mybir.AxisListType.X, op=Alu.add)
            # stage incrementally so the AllReduce's input is ready in DRAM
            # the moment the collective launches
            nc.sync.dma_start(ccdst_s, dst_s)
            nc.sync.dma_start(ccdst_q, dst_q)

        def emit_cumsum_steps(j):
            for t in (2 * j, 2 * j + 1):
                if t >= T:
                    continue
                if t % TPB == 0:
                    cur_chunk[0] = uftp.tile([128, TPB * BSH], f32,
                                             tag="uftc", name=f"uftc{it}_{t}")
                src = (G[0:BSH, (t // 2) * D:(t // 2 + 1) * D] if t % 2 == 0
                       else G2[0:BSH, (t // 2) * D:(t // 2 + 1) * D])
                pf = pfs[t % 2]
                if t == 0:
                    nc.vector.tensor_copy(pf[:], src)
                else:
                    nc.vector.tensor_tensor(out=pf[:], in0=pfs[(t - 1) % 2][:],
                                            in1=src, op=Alu.add)
                ti = t % TPB
                nc.tensor.matmul(cur_chunk[0][:, ti * BSH:(ti + 1) * BSH],
                                 lhsT=pf[:],
                                 rhs=rdiag_s[:, t * BSH:(t + 1) * BSH],
                                 start=True, stop=True)
                if (t + 1) % TPB == 0:
                    stats_chunk(t + 1 - TPB, t + 1)
                elif t == T - 1:
                    stats_chunk(T - T % TPB, T)

        # cumsum for a given j is only emitted once its G2 chunk's copy has
        # been emitted: Tile's dep tracking is emission-ordered, and a read
        # emitted before its writer gets no RAW edge (reads stale data).
        for j in range(TH):
            nc.gpsimd.indirect_dma_start(
                out=G[:, j * D:(j + 1) * D], out_offset=None, in_=emb,
                in_offset=bass.IndirectOffsetOnAxis(ap=offs_s[:, j:j + 1], axis=0),
            )
            if (j + 1) % GCH == 0:
                lo = (j + 1 - GCH) * D
                nc.sync.dma_start(G2[:, lo:(j + 1) * D], G[BSH:128, lo:(j + 1) * D])
                for jj in range(j + 1 - GCH, j + 1):
                    emit_cumsum_steps(jj)
            if j == A_LAUNCH_J and comm in ("rdma", "cc"):
                nc.gpsimd.collective_compute(
                    "AllReduce", Alu.add, replica_groups=groups,
                    ins=[cc_inA[:]], outs=[cc_outA[:]])
        if TH % GCH:
            lo = (TH - TH % GCH) * D
            nc.sync.dma_start(G2[:, lo:TH * D], G[BSH:128, lo:TH * D])
            for jj in range(TH - TH % GCH, TH):
                emit_cumsum_steps(jj)
        if comm in ("rdma", "cc"):
            nc.gpsimd.collective_compute(
                "AllReduce", Alu.add, replica_groups=groups,
                ins=[cc_inB[:]], outs=[cc_outB[:]])
        else:
            nc.sync.dma_start(cc_outA[:], cc_inA[:])
            nc.sync.dma_start(cc_outB[:], cc_inB[:])

        gstatsA = work.tile([D, 2 * NA], f32, name=f"gstatsA{it}")
        nc.sync.dma_start(gstatsA[:], cc_outA[:])
        gstatsB = work.tile([D, 2 * NB_], f32, name=f"gstatsB{it}")
        nc.sync.dma_start(gstatsB[:], cc_outB[:])

        # ---- BN affine params: h_t = x*s2_t + b2_t  (pre-divided by TAU)
        bh = work.tile([D, 1], f32, name=f"bh{it}")
        nc.vector.tensor_scalar(out=bh[:], in0=pp_s[:, 1:2],
                                scalar1=1.0 / TAU, scalar2=None, op0=Alu.mult)
        s2 = work.tile([D, T], f32, name=f"s2{it}")
        b2 = work.tile([D, T], f32, name=f"b2{it}")

        def emit_params(gst, n, col0, tag):
            mean = work.tile([D, n], f32, name=f"mean{tag}{it}")
            nc.vector.tensor_scalar(out=mean[:], in0=gst[:, 0:n],
                                    scalar1=1.0 / B, scalar2=None, op0=Alu.mult)
            ex2 = work.tile([D, n], f32, name=f"ex2{tag}{it}")
            nc.vector.tensor_scalar(out=ex2[:], in0=gst[:, n:2 * n],
                                    scalar1=1.0 / B, scalar2=None, op0=Alu.mult)
            var = work.tile([D, n], f32, name=f"var{tag}{it}")
            nc.vector.tensor_tensor(out=var[:], in0=mean[:], in1=mean[:],
                                    op=Alu.mult)
            nc.vector.tensor_tensor(out=var[:], in0=ex2[:], in1=var[:],
                                    op=Alu.subtract)
            std = work.tile([D, n], f32, name=f"std{tag}{it}")
            nc.scalar.activation(std[:], var[:], Act.Sqrt, bias=eps_t[:, 0:1])
            inv = work.tile([D, n], f32, name=f"inv{tag}{it}")
            nc.vector.reciprocal(inv[:], std[:])
            s2s = s2[:, col0:col0 + n]
            nc.vector.tensor_scalar(out=s2s, in0=inv[:], scalar1=pp_s[:, 0:1],
                                    scalar2=1.0 / TAU, op0=Alu.mult,
                                    op1=Alu.mult)
            ms = work.tile([D, n], f32, name=f"ms{tag}{it}")
            nc.vector.tensor_tensor(out=ms[:], in0=mean[:], in1=s2s,
                                    op=Alu.mult)
            nc.vector.scalar_tensor_tensor(
                out=b2[:, col0:col0 + n], in0=ms[:], scalar=-1.0,
                in1=bh[:, 0:1].to_broadcast((D, n)), op0=Alu.mult, op1=Alu.add)

        emit_params(gstatsA, NA, 0, "A")
        emit_params(gstatsB, NB_, TSPLIT, "B")

        # ---- LIF recurrence on the pre-reset voltage w:
        #   s_t = [w_t >= 1];  w_{t+1} = (w_t - s_t)/2 + h_{t+1}
        # h precomputed for every t into one big tile so the Act engine runs
        # ahead of the serial DVE chain; spikes accumulated on the idle PE
        # via identity-matmul PSUM accumulation (kills the tail reduce).
        h = con.tile([128, T * BSH], f32, name=f"h{it}")
        for t in range(T):
            nc.scalar.activation(h[:, t * BSH:(t + 1) * BSH],
                                 uFT_sb[:, t * BSH:(t + 1) * BSH],
                                 Act.Identity, scale=s2[:, t:t + 1],
                                 bias=b2[:, t:t + 1])

        w = work.tile([128, BSH], f32, name=f"w{it}")
        q = work.tile([128, BSH], f32, name=f"q{it}")
        spk = con.tile([128, T * BSH], f32, name=f"spk{it}")
        for t in range(T):
            ht = h[:, t * BSH:(t + 1) * BSH]
            if t == 0:
                nc.vector.tensor_copy(w[:], ht)
            else:
                nc.vector.scalar_tensor_tensor(
                    out=q[:], in0=w[:], scalar=1.0 / TAU, in1=ht,
                    op0=Alu.mult, op1=Alu.add)
                nc.vector.scalar_tensor_tensor(
                    out=w[:], in0=spk[:, (t - 1) * BSH:t * BSH],
                    scalar=-V_TH / TAU, in1=q[:], op0=Alu.mult, op1=Alu.add)
            nc.vector.tensor_scalar(out=spk[:, t * BSH:(t + 1) * BSH],
                                    in0=w[:], scalar1=V_TH, scalar2=None,
                                    op0=Alu.is_ge)
            nc.tensor.matmul(acc[:], lhsT=i128_s[:],
                             rhs=spk[:, t * BSH:(t + 1) * BSH],
                             start=(t == 0), stop=(t == T - 1))

        # uo payload [128, 65]: cols 0..63 = mean spikes (bf16), col 64 row 0
        # = this core's id tag (receivers recover the sender->slot map from it)
        uo = work.tile([128, WSLOT], bf16, name=f"uo{it}")
        uo_w = nc.scalar.activation(uo[:, 0:BSH], acc[:], Act.Identity,
                                    scale=1.0 / T)
        nc.vector.tensor_copy(uo[0:1, BSH:BSH + 1], pid_s[:])
        if comm in ("rdma", "rdma_nostats") and it > 0:
            cs.post_waits.append((nc.scalar, uo_w.ins.name, cs.lsem,
                                  16 * NCORES * it))

        # ---- uo exchange: 8 single-dest remote broadcasts (slot k -> peer
        # me^k), descriptors prepped on the idle Pool engine during the LIF,
        # fired by one trigger once uo is ready. Lands directly in peers'
        # SBUF recv tile -- no DRAM round trip, no collective.
        lhsT = work.tile([D, B], bf16, name=f"lhsT{it}")
        if comm == "cc":
            # collective AllGather fallback (v0-style): SBUF->DRAM->AG->SBUF
            ag_in = dr.tile([D, BSH], bf16, name=f"ag_in{it}")
            ag_out = dr.tile([NCORES * D, BSH], bf16, addr_space="Shared",
                             name=f"ag_out{it}")
            nc.sync.dma_start(ag_in[:], uo[:, 0:BSH])
            nc.gpsimd.collective_compute(
                "AllGather", Alu.bypass, replica_groups=groups,
                ins=[ag_in[:]], outs=[ag_out[:]])
            nc.sync.dma_start(
                lhsT[:].rearrange("p (c b) -> p c b", c=NCORES),
                ag_out[:].rearrange("(c p) b -> p c b", c=NCORES))
            for k in range(NCORES):
                nc.sync.dma_start(perm[0:1, k:k + 1], uo[0:1, BSH:BSH + 1])
        elif comm in ("rdma", "rdma_nostats"):
            for k in range(NCORES):
                rdests = [None] * NCORES
                rdests[k] = (0, k)
                nc.gpsimd.remote_dma_broadcast(
                    out_ap=recv[:, k * WSLOT:(k + 1) * WSLOT],
                    in_ap=uo[:, 0:WSLOT], remote_sem=cs.rsem,
                    local_sem=cs.lsem, rdests=rdests, queue_num=1)
            trig = nc.gpsimd.trigger_dma(count=None, queue_num=1)
            cs.post_waits.append((nc.gpsimd, trig.ins.name, cs.barrier_sem,
                                  cs.barrier_val))
            rthresh = 2 * NCORES * (it + 1)
            # assemble lhsT from the recv slots (data cols only); first copy
            # per engine carries the arrival wait
            for k in range(NCORES):
                src = recv[:, k * WSLOT:k * WSLOT + BSH]
                dst = lhsT[:, k * BSH:(k + 1) * BSH]
                if k % 2 == 0:
                    cp = nc.vector.tensor_copy(dst, src)
                    eng = nc.vector
                else:
                    cp = nc.scalar.activation(dst, src, Act.Copy)
                    eng = nc.scalar
                if k < 2:
                    cs.post_waits.append((eng, cp.ins.name, cs.rsem, rthresh))
            for k in range(NCORES):
                pdma = nc.sync.dma_start(
                    perm[0:1, k:k + 1],
                    recv[0:1, k * WSLOT + BSH:k * WSLOT + BSH + 1])
                if k == 0:
                    cs.post_waits.append((nc.sync, pdma.ins.name, cs.rsem,
                                          rthresh))
        else:
            # dev-only fallback (TimelineSim): replicate own uo into all slots
            for k in range(NCORES):
                nc.vector.tensor_copy(lhsT[:, k * BSH:(k + 1) * BSH],
                                      uo[:, 0:BSH])
                nc.sync.dma_start(perm[0:1, k:k + 1],
                                  uo[0:1, BSH:BSH + 1])

        if DEBUG_DUMP and it == 0:
            nc.sync.dma_start(aps["dbg_h"], h[:])
            nc.sync.dma_start(aps["dbg_spk"], spk[:])
            nc.sync.dma_start(aps["dbg_lhsT"], lhsT[:])
            dbg_uo_s = work.tile([128, WSLOT], mybir.dt.float32,
                                 name=f"dbg_uo_s{it}")
            nc.vector.tensor_copy(dbg_uo_s[:], uo[:])
            nc.sync.dma_start(aps["dbg_uo"], dbg_uo_s[:])
            nc.sync.dma_start(aps["dbg_uft"], uFT_sb[:])
            nc.sync.dma_start(aps["dbg_s2"], s2[:])
            nc.sync.dma_start(aps["dbg_b2"], b2[:])

        # ---- scores matmul, vocab-sharded. Evict 4 psum blocks into one wide
        # staging tile per out-DMA (HWDGE descriptor-gen bound otherwise);
        # evictions cycle 1:2 over DVE:Act (DVE is the busier engine).
        NBLK = 512
        GRP = 4
        with tc.tile_pool(name=f"psB{it}", bufs=5, space="PSUM") as psB, \
             tc.tile_pool(name=f"ost{it}", bufs=6) as ostage:
            k2 = 0
            for m in range(B // 128):
                n = 0
                while n < VSH // NBLK:
                    g = min(GRP, VSH // NBLK - n)
                    ot = ostage.tile([128, GRP * NBLK], bf16, tag="ot",
                                     name=f"ot{it}_{m}_{n}")
                    for i in range(g):
                        mm = psB.tile([128, NBLK], f32, tag="mm",
                                      name=f"mm{it}_{k2}")
                        nc.tensor.matmul(
                            mm[:], lhsT=lhsT[:, m * 128:(m + 1) * 128],
                            rhs=embT_s[:, (n + i) * NBLK:(n + i + 1) * NBLK],
                            start=True, stop=True)
                        dst = ot[:, i * NBLK:(i + 1) * NBLK]
                        # 1:2 DVE:Act eviction split -- keeps DVE light so the
                        # next iteration's cumsum isn't queued far behind
                        if k2 % 3 == 0:
                            nc.vector.tensor_copy(dst, mm[:])
                        else:
                            nc.scalar.activation(dst, mm[:], Act.Copy)
                        k2 += 1
                    nc.sync.dma_start(
                        out[m * 128:(m + 1) * 128,
                            n * NBLK:(n + g) * NBLK], ot[:, 0:g * NBLK])
                    n += g


def _build(unroll=1, comm=None, num_devices=NCORES):
    if comm is None:
        comm = COMM_MODE
    import concourse.tile as tile
    from concourse import bacc, mybir

    f32 = mybir.dt.float32
    bf16 = mybir.dt.bfloat16
    i32 = mybir.dt.int32

    nc = bacc.Bacc("TRN2", target_bir_lowering=False, debug=False,
                   num_devices=num_devices, num_swdge_queues=2)
    aps = {
        "emb": nc.dram_tensor("emb", [N_ITEMS, D], f32, kind="ExternalInput").ap(),
        "embT": nc.dram_tensor("embT", [D, VSH], bf16, kind="ExternalInput").ap(),
        "offs": nc.dram_tensor("offs", [128, TH], i32, kind="ExternalInput").ap(),
        "rdiag": nc.dram_tensor("rdiag", [BSH, T * BSH], f32,
                                kind="ExternalInput").ap(),
        "pp": nc.dram_tensor("pp", [D, 2], f32, kind="ExternalInput").ap(),
        "i128": nc.dram_tensor("i128", [128, 128], f32, kind="ExternalInput").ap(),
        "out": nc.dram_tensor("out", [B, VSH], bf16, kind="ExternalOutput").ap(),
        "perm": nc.dram_tensor("perm", [1, NCORES], bf16,
                               kind="ExternalOutput").ap(),
    }
    if DEBUG_DUMP:
        aps["dbg_h"] = nc.dram_tensor("dbg_h", [128, T * BSH], f32,
                                      kind="ExternalOutput").ap()
        aps["dbg_spk"] = nc.dram_tensor("dbg_spk", [128, T * BSH], f32,
                                        kind="ExternalOutput").ap()
        aps["dbg_lhsT"] = nc.dram_tensor("dbg_lhsT", [D, B], bf16,
                                         kind="ExternalOutput").ap()
        aps["dbg_uo"] = nc.dram_tensor("dbg_uo", [128, WSLOT], f32,
                                       kind="ExternalOutput").ap()
        aps["dbg_uft"] = nc.dram_tensor("dbg_uft", [128, T * BSH], f32,
                                        kind="ExternalOutput").ap()
        aps["dbg_s2"] = nc.dram_tensor("dbg_s2", [D, T], f32,
                                       kind="ExternalOutput").ap()
        aps["dbg_b2"] = nc.dram_tensor("dbg_b2", [D, T], f32,
                                       kind="ExternalOutput").ap()
    cs = None
    if comm in ("rdma", "rdma_nostats"):
        groups = [list(range(num_devices))]
        nc._bir_kernel_barrier_sem_replica_groups.extend(set(g) for g in groups)
        cs = _Comm(nc)
    u32 = mybir.dt.uint32
    with tile.TileContext(nc) as tc:
        with tc.tile_pool(name="commp", bufs=1) as commp:
            recvs = [commp.tile([128, NCORES * WSLOT],
                                mybir.dt.bfloat16, name=f"recv{p}")
                     for p in range(2)]
            cons = {
                "offs_s": commp.tile([128, TH], i32, name="c_offs"),
                "rdiag_s": commp.tile([B // NCORES, T * B // NCORES], f32,
                                      name="c_rdiag"),
                "pp_s": commp.tile([D, 2], f32, name="c_pp"),
                "i128_s": commp.tile([128, 128], f32, name="c_i128"),
                "pid_s": commp.tile([1, 1], u32, name="c_pid"),
                "eps_t": commp.tile([D, 1], f32, name="c_eps"),
                "embT_s": commp.tile([D, VSH], bf16, name="c_embT"),
            }
            nc.sync.dma_start(cons["offs_s"][:], aps["offs"])
            nc.sync.dma_start(cons["rdiag_s"][:], aps["rdiag"])
            nc.sync.dma_start(cons["pp_s"][:], aps["pp"])
            nc.sync.dma_start(cons["i128_s"][:], aps["i128"])
            nc.sync.dma_start(cons["pid_s"][:],
                              nc.partition_id_tensor[0:1, 0:1])
            nc.vector.memset(cons["eps_t"][:], 1e-5)
            for q in range(4):
                nc.scalar.dma_start(
                    cons["embT_s"][:, q * (VSH // 4):(q + 1) * (VSH // 4)],
                    aps["embT"][:, q * (VSH // 4):(q + 1) * (VSH // 4)])
            for it in range(unroll):
                _emit_iteration(nc, tc, aps, cs, recvs[it % 2], cons, it=it,
                                comm=comm)
    if cs is not None and cs.post_waits:
        _attach_post_waits(nc, cs.post_waits)
    nc.compile()
    return nc


def _prep_inputs(seq, lengths, emb_table, gamma, beta):
    seq = np.asarray(seq)
    lengths = np.asarray(lengths)
    emb_table = np.asarray(emb_table, dtype=np.float32)
    gamma = np.asarray(gamma, dtype=np.float32)
    beta = np.asarray(beta, dtype=np.float32)

    emb_full = emb_table.copy()
    emb_full[0, :] = 0.0

    tt = np.arange(1, T + 1, dtype=np.float64)[None, :]
    denom = np.minimum(tt, lengths.astype(np.float64)[:, None])
    rd = (1.0 / denom).astype(np.float32)                      # [B, T]

    embT_full = np.zeros((D, NCORES * VSH), dtype=ml_dtypes.bfloat16)
    embT_full[:, :N_ITEMS] = emb_full.T.astype(ml_dtypes.bfloat16)

    pp = np.stack([gamma, beta], axis=1).astype(np.float32)    # [128, 2]
    i128 = np.eye(128, dtype=np.float32)

    in_maps = []
    for c in range(NCORES):
        sl = slice(c * BSH, (c + 1) * BSH)
        seq_c = seq[sl].astype(np.int32)                       # [64, 50]
        offs_c = np.concatenate([seq_c[:, 0::2], seq_c[:, 1::2]], axis=0)
        offs_c = np.ascontiguousarray(offs_c)                  # [128, 25]
        rd_c = rd[sl]                                          # [64, 50]
        r3 = np.zeros((BSH, T, BSH), dtype=np.float32)
        for b in range(BSH):
            r3[b, :, b] = rd_c[b]
        rdiag_c = np.ascontiguousarray(r3.reshape(BSH, T * BSH))
        embT_c = np.ascontiguousarray(embT_full[:, c * VSH:(c + 1) * VSH])
        in_maps.append({
            "emb": emb_full, "embT": embT_c, "offs": offs_c,
            "rdiag": rdiag_c, "pp": pp, "i128": i128,
        })
    return in_maps


def _assemble(results, use_perm=True):
    """Reorder each core's output rows using its slot->sender tag row, then
    concatenate vocab shards."""
    scores = np.empty((B, NCORES * VSH), dtype=np.float32)
    for c in range(NCORES):
        if use_perm:
            perm = np.asarray(results[c]["perm"][0], dtype=np.float32).astype(int)
            assert sorted(perm.tolist()) == list(range(NCORES)), (
                f"core {c}: bad uo-exchange tags {perm}")
        else:
            perm = np.arange(NCORES)
        oc = np.asarray(results[c]["out"], dtype=np.float32)
        dst = scores[:, c * VSH:(c + 1) * VSH]
        for k in range(NCORES):
            dst[perm[k] * BSH:(perm[k] + 1) * BSH] = oc[k * BSH:(k + 1) * BSH]
    return np.ascontiguousarray(scores[:, :N_ITEMS])


def _cached_runner(nc, reps_key):
    """Build (once) a jitted shard_map runner with device-resident input
    placement for repeated timed executions of nc's single bass_exec."""
    import jax
    from jax.sharding import Mesh, PartitionSpec
    from jax.experimental.shard_map import shard_map
    from concourse import mybir
    from concourse.bass2jax import (_bass_exec_p, partition_id_tensor,
                                    install_neuronx_cc_hook)
    install_neuronx_cc_hook()

    in_names, out_names, out_avals = [], [], []
    for alloc in nc.m.functions[0].allocations:
        if not isinstance(alloc, mybir.MemoryLocationSet):
            continue
        name = alloc.memorylocations[0].name
        if alloc.kind == "ExternalInput":
            if nc.partition_id_tensor is None or name != nc.partition_id_tensor.name:
                in_names.append(name)
        elif alloc.kind == "ExternalOutput":
            out_names.append(name)
            out_avals.append(jax.core.ShapedArray(
                tuple(alloc.tensor_shape), mybir.dt.np(alloc.dtype)))
    n_params = len(in_names)
    all_in = list(in_names) + list(out_names)
    if nc.partition_id_tensor is not None:
        all_in.append(nc.partition_id_tensor.name)

    def _body(*args):
        operands = list(args)
        if nc.partition_id_tensor is not None:
            operands.append(partition_id_tensor())
        return tuple(_bass_exec_p.bind(
            *operands, out_avals=tuple(out_avals), in_names=tuple(all_in),
            out_names=tuple(out_names), lowering_input_output_aliases=(),
            sim_require_finite=True, sim_require_nnan=True, nc=nc))

    mesh = Mesh(np.asarray(jax.devices()[:NCORES]), ("core",))
    n_outs = len(out_names)
    f = jax.jit(shard_map(
        _body, mesh=mesh,
        in_specs=(PartitionSpec("core"),) * (n_params + n_outs),
        out_specs=(PartitionSpec("core"),) * n_outs, check_rep=False))
    return f, in_names, out_avals


def benchmark(seq, lengths, emb_table, gamma, beta, unroll=16, pairs=30):
    """Estimate per-iteration device time via the slope between a 1x and a
    Kx-unrolled build of the same program (identical I/O staging costs).
    Executions are interleaved in (1x, Kx) pairs so axon-terminal drift
    cancels; the median pair-difference / (K-1) is the per-iteration time."""
    import jax, time, statistics
    in_maps = _prep_inputs(seq, lengths, emb_table, gamma, beta)
    if "nc" not in _CACHE:
        _CACHE["nc"] = _build()
    key = f"nc{unroll}"
    if key not in _CACHE:
        _CACHE[key] = _build(unroll=unroll)

    runners = []
    for nc in (_CACHE["nc"], _CACHE[key]):
        f, in_names, out_avals = _cached_runner(nc, None)
        per_core = [[np.asarray(m[nm]) for nm in in_names] for m in in_maps]
        ci = [jax.device_put(np.concatenate(
            [per_core[c][i] for c in range(NCORES)], axis=0))
            for i in range(len(in_names))]
        cz = [jax.device_put(np.zeros((NCORES * a.shape[0], *a.shape[1:]),
                                      a.dtype)) for a in out_avals]
        outx = f(*ci, *cz)
        jax.block_until_ready(outx)
        runners.append((f, ci, cz))

    def run_one(i):
        f, ci, cz = runners[i]
        t0 = time.perf_counter()
        outx = f(*ci, *cz)
        jax.block_until_ready(outx)
        return time.perf_counter() - t0

    diffs = []
    for _ in range(pairs):
        a = run_one(0)
        b = run_one(1)
        diffs.append(b - a)
    diffs.sort()
    med = diffs[len(diffs) // 2]
    per_iter_ns = med / (unroll - 1) * 1e9
    return per_iter_ns, {
        "median_diff_ms": med * 1e3,
        "mean_diff_ms": statistics.mean(diffs) * 1e3,
        "stdev_ms": statistics.stdev(diffs) * 1e3,
        "unroll": unroll, "pairs": pairs,
    }


def kernel(seq, lengths, emb_table, gamma, beta, trace=False):
    global LAST_EXEC_NS, LAST_RESULTS
    from concourse.bass_utils import run_bass_kernel_spmd

    if "nc" not in _CACHE:
        _CACHE["nc"] = _build()
    nc = _CACHE["nc"]

    in_maps = _prep_inputs(seq, lengths, emb_table, gamma, beta)
    res = run_bass_kernel_spmd(nc, in_maps, core_ids=list(range(NCORES)))
    LAST_EXEC_NS = res.exec_time_ns
    LAST_RESULTS = res
    return _assemble(res.results, use_perm=(COMM_MODE == "rdma"))
